# revision 23
# baseline (speedup 1.0000x reference)
"""GCN encoder (edge-wise message passing) on 8 Trainium2 NeuronCores.

Strategy (dst-range sharding):
  - Host: sort edges by dst, shard by dst-range (core r owns nodes
    [r*NLOC, (r+1)*NLOC)), group edges into 128-node windows, pad each
    (window, src-half) group to 128-multiples. Degree / index prep on host.
  - Device: BN stats via ACT-accumulate + tiny AllReduce, folded into W_i.
    Pre-pass computes f_e (feature-major) once, materializes the
    loop-invariant per-edge base = f_e @ Wh_mid + p*w_p to HBM bf16 in a
    partition-contiguous layout, and performs the iter-0 scatter.
    Each iteration: batched per-superwindow dma_gathers pull g_s[src] rows
    from the AllGathered global src-table and g_d[dst] rows from the local
    dst-table; eh = relu(base + g_s + g_d) via two in-place DVE adds + one
    ACT relu; scatter-mean via one-hot matmul into PSUM per 128-node
    window (the one-hot S matrices are built in bulk with stride-0
    broadcast is_equal ops, split across DVE and GPSIMD); then next tables
    g_s|g_d = h @ [Wh_src|Wh_dst] (+b_h), src half AllGathered.
"""
import sys
sys.path.insert(0, "/opt/trn_rl_repo")

import numpy as np
import ml_dtypes
from contextlib import ExitStack

from concourse import bass, bacc, mybir, tile, masks
from concourse.bass_utils import run_bass_kernel_spmd

f32 = mybir.dt.float32
bf16 = mybir.dt.bfloat16
i16 = mybir.dt.int16
i32 = mybir.dt.int32
AO = mybir.AluOpType
AF = mybir.ActivationFunctionType

NCORES = 8
DEPTH = 3
EPS = 1e-5
GW = 4            # windows per superwindow
USE_RDMA = False   # replace table AllGather with remote_dma_broadcast rounds
STAT_SLICE = 1024
S_DVE_BLOCKS = 38   # per-sw S-build blocks on DVE (rest on GPSIMD), /64ths

bfl = ml_dtypes.bfloat16


def _ru(x, m):
    return (x + m - 1) // m * m


class Plan:
    """Host-side preprocessing: sharding, sorting, padding, index layout."""

    def __init__(self, src, dst, N):
        E = src.shape[0]
        self.N, self.E = N, E
        self.NLOC = (N + NCORES - 1) // NCORES
        self.NWIN = (self.NLOC + 127) // 128
        self.NLOCP = self.NWIN * 128
        self.NGLOB = NCORES * self.NLOCP
        # rdma allgather: pad per-core table rows to 4 equal rounds
        self.NWP = _ru(self.NWIN, 4)
        self.NROW = self.NWP * 128
        self.NGLOBR = NCORES * self.NROW
        # src-half split: largest rank-multiple of NROW that fits int16
        self.SPLIT = min((32768 // self.NROW) * self.NROW, self.NGLOBR)
        assert self.NGLOBR - self.SPLIT < 32768

        owner = dst // self.NLOC
        local = dst - owner * self.NLOC
        win = local >> 7
        self.ohval_all = (local & 127).astype(np.float32)
        srcrow = (src // self.NLOC) * self.NROW + (src % self.NLOC)
        half = (srcrow >= self.SPLIT).astype(np.int64)
        self.srcrow, self.local, self.owner, self.win, self.half = (
            srcrow, local, owner, win, half)

        key = (owner * self.NWIN + win) * 2 + half
        self.order = np.argsort(key, kind="stable")
        cnt = np.bincount(key, minlength=NCORES * self.NWIN * 2)
        cnt = cnt.reshape(NCORES, self.NWIN, 2)
        self.capA = np.maximum(_ru(cnt[:, :, 0].max(0), 128), 128)
        self.capB = _ru(cnt[:, :, 1].max(0), 128)
        self.cnt = cnt

        # superwindows
        self.NSW = (self.NWIN + GW - 1) // GW
        self.sw_windows = [list(range(s * GW, min((s + 1) * GW, self.NWIN)))
                           for s in range(self.NSW)]
        # slot layout: per sw, [A_w0 B_w0 | A_w1 B_w1 | ...] so each
        # window's chunks are contiguous (single open psum group at a time)
        self.slotA = np.zeros(self.NWIN, np.int64)   # slot offset of A group
        self.slotB = np.zeros(self.NWIN, np.int64)
        self.sw_off = np.zeros(self.NSW + 1, np.int64)
        off = 0
        for s, ws in enumerate(self.sw_windows):
            self.sw_off[s] = off
            a = off
            for w in ws:
                self.slotA[w] = a
                a += self.capA[w]
                self.slotB[w] = a
                a += self.capB[w]
            off = a
        self.sw_off[self.NSW] = off
        self.ES = int(off)
        self.sw_capA = [int(sum(self.capA[w] for w in ws))
                        for ws in self.sw_windows]
        self.sw_capB = [int(sum(self.capB[w] for w in ws))
                        for ws in self.sw_windows]
        self.sw_cap = [a + b for a, b in zip(self.sw_capA, self.sw_capB)]
        self.EMAX4 = _ru(max(int((owner == r).sum()) for r in range(NCORES)), 512)
        self.Q4 = self.EMAX4 // 4

    def signature(self):
        return (self.N, self.E, tuple(self.capA), tuple(self.capB))


def _host_inputs(plan, e, p, src, dst):
    """Build the per-core input arrays."""
    NLOC, NWIN, ES = plan.NLOC, plan.NWIN, plan.ES
    order, cnt = plan.order, plan.cnt
    deg = np.maximum(np.bincount(dst, minlength=plan.N), 1).astype(np.float32)
    invd = 1.0 / deg

    in_maps = []
    pos = 0
    # order slices per (r, w, h) in key order
    slices = {}
    for r in range(NCORES):
        for w in range(NWIN):
            for h in range(2):
                c = int(cnt[r, w, h])
                slices[(r, w, h)] = order[pos:pos + c]
                pos += c
    assert pos == plan.E

    for r in range(NCORES):
        efm = np.zeros((34, ES), np.float32)
        efm[32, :] = 1.0
        gsx = np.zeros(ES, np.int16)
        gdx = np.zeros(ES, np.int16)
        ohv = np.full(ES, -5.0, np.float32)
        for w in range(NWIN):
            for h, base_slot in ((0, plan.slotA[w]), (1, plan.slotB[w])):
                idx = slices[(r, w, h)]
                n = idx.shape[0]
                sl = slice(base_slot, base_slot + n)
                efm[0:32, sl] = e[idx].T
                efm[33, sl] = p[idx, 0]
                gsx[sl] = plan.srcrow[idx] - (plan.SPLIT if h else 0)
                gdx[sl] = plan.local[idx]
                ohv[sl] = plan.ohval_all[idx]

        # wrap idxs per superwindow: [16, cap/16] replicated x8
        def wrap(arr):
            out = np.zeros((128, ES // 16), np.int16)
            for s in range(plan.NSW):
                o0, o1 = int(plan.sw_off[s]), int(plan.sw_off[s + 1])
                seg = arr[o0:o1].reshape(-1, 16).T
                out[:, o0 // 16:o1 // 16] = np.tile(seg, (8, 1))
            return out

        soh = ohv.reshape(-1, 128).T.copy()  # [128, ES//128]
        ivl = np.ones(plan.NLOCP, np.float32)
        lo, hi = r * NLOC, min((r + 1) * NLOC, plan.N)
        ivl[:hi - lo] = invd[lo:hi]
        invdeg = ivl.reshape(NWIN, 128).T.copy()  # [128, NWIN]

        mask = plan.owner == np.int64(r)
        er = e[mask]
        epad = np.zeros((plan.EMAX4, 32), np.float32)
        epad[:er.shape[0]] = er
        e4 = epad.reshape(4, plan.Q4, 32).transpose(0, 2, 1).reshape(128, plan.Q4)

        in_maps.append({
            "efm": efm.astype(bfl),
            "gs_idx": wrap(gsx),
            "gd_idx": wrap(gdx),
            "sohb": soh,
            "invdeg": invdeg,
            "e4": e4.astype(bfl),
        })
    return in_maps


def _weight_inputs(plan, gamma, beta, W_i, b_i, W_h, b_h):
    OUT = W_i.shape[1]
    whmid = np.zeros((OUT, 128), np.float32)
    whmid[:, :OUT] = W_h[OUT:2 * OUT]
    wp2 = np.zeros((2, 128), np.float32)
    wp2[1, :OUT] = W_h[2 * OUT]
    whsd = np.zeros((OUT + 1, 256), np.float32)
    whsd[:OUT, 0:OUT] = W_h[0:OUT]
    whsd[:OUT, 128:128 + OUT] = W_h[2 * OUT + 1:3 * OUT + 1]
    whsd[OUT, 128:128 + OUT] = b_h
    return {
        "W_i": W_i.astype(np.float32),
        "b_i": b_i.reshape(OUT, 1).astype(np.float32),
        "gamma": gamma.reshape(32, 1).astype(np.float32),
        "beta": beta.reshape(32, 1).astype(np.float32),
        "whmid": whmid.astype(bfl),
        "wp2": wp2.astype(bfl),
        "whsd": whsd.astype(bfl),
    }


def _build(plan, OUT):
    """Build + compile the SPMD Bass program for this plan."""
    NWIN, NSW, ES = plan.NWIN, plan.NSW, plan.ES
    NLOCP, NGLOB, SPLIT = plan.NLOCP, plan.NGLOB, plan.SPLIT
    IN = 32

    nc = bacc.Bacc("TRN2", target_bir_lowering=False, debug=False,
                   num_devices=NCORES)

    efm = nc.dram_tensor("efm", [34, ES], bf16, kind="ExternalInput")
    gs_idx = nc.dram_tensor("gs_idx", [128, ES // 16], i16, kind="ExternalInput")
    gd_idx = nc.dram_tensor("gd_idx", [128, ES // 16], i16, kind="ExternalInput")
    sohb = nc.dram_tensor("sohb", [128, ES // 128], f32, kind="ExternalInput")
    invdeg = nc.dram_tensor("invdeg", [128, NWIN], f32, kind="ExternalInput")
    e4 = nc.dram_tensor("e4", [128, plan.Q4], bf16, kind="ExternalInput")
    W_i = nc.dram_tensor("W_i", [IN, OUT], f32, kind="ExternalInput")
    b_i = nc.dram_tensor("b_i", [OUT, 1], f32, kind="ExternalInput")
    gamma = nc.dram_tensor("gamma", [IN, 1], f32, kind="ExternalInput")
    beta = nc.dram_tensor("beta", [IN, 1], f32, kind="ExternalInput")
    whmid = nc.dram_tensor("whmid", [OUT, 128], bf16, kind="ExternalInput")
    wp2 = nc.dram_tensor("wp2", [2, 128], bf16, kind="ExternalInput")
    whsd = nc.dram_tensor("whsd", [OUT + 1, 256], bf16, kind="ExternalInput")

    out_fn = nc.dram_tensor("out_fn", [NLOCP, OUT], f32, kind="ExternalOutput")
    out_h = nc.dram_tensor("out_h", [NLOCP, OUT], f32, kind="ExternalOutput")

    inv_E = 1.0 / plan.E

    with tile.TileContext(nc) as tc:
        with ExitStack() as ctx:
            cpool = ctx.enter_context(tc.tile_pool(name="cpool", bufs=1))
            pool = ctx.enter_context(tc.tile_pool(name="pool", bufs=2))
            spool = ctx.enter_context(tc.tile_pool(name="spool", bufs=2))
            psum = ctx.enter_context(tc.tile_pool(name="psum", bufs=2,
                                                  space="PSUM"))
            dram = ctx.enter_context(tc.tile_pool(name="dram", bufs=1,
                                                  space="DRAM"))

            # ---- constants ----
            iota_i = cpool.tile([128, 128], i32)
            nc.gpsimd.iota(iota_i[:], pattern=[[1, 128]], base=0,
                           channel_multiplier=0)
            iota_b = cpool.tile([128, 128], bf16)
            nc.vector.tensor_copy(iota_b[:], iota_i[:])
            ident = cpool.tile([128, 128], f32)
            masks.make_identity(nc, ident[:])

            whmid_t = cpool.tile([OUT, 128], bf16)
            nc.sync.dma_start(whmid_t[:], whmid[:])
            wp2_t = cpool.tile([34, 128], bf16)
            nc.sync.dma_start(wp2_t[32:34, :], wp2[:])
            whsd_t = cpool.tile([OUT + 1, 256], bf16)
            nc.sync.dma_start(whsd_t[:], whsd[:])
            invdeg_t = cpool.tile([128, NWIN], f32)
            nc.sync.dma_start(invdeg_t[:], invdeg[:])

            # ---- BN stats: per-core partial sums of e, e^2 ----
            nsl = (plan.Q4 + STAT_SLICE - 1) // STAT_SLICE
            parts = cpool.tile([128, 2 * nsl], f32)
            for s in range(nsl):
                c0, c1 = s * STAT_SLICE, min((s + 1) * STAT_SLICE, plan.Q4)
                esl = spool.tile([128, STAT_SLICE], bf16, tag="esl")
                nc.sync.dma_start(esl[:, :c1 - c0], e4[:, c0:c1])
                junk = spool.tile([128, STAT_SLICE], bf16, tag="junk")
                nc.scalar.activation(junk[:, :c1 - c0], esl[:, :c1 - c0],
                                     AF.Copy, accum_out=parts[:, s:s + 1])
                nc.scalar.activation(junk[:, :c1 - c0], esl[:, :c1 - c0],
                                     AF.Square,
                                     accum_out=parts[:, nsl + s:nsl + s + 1])
            sums = cpool.tile([128, 2], f32)
            junk2 = cpool.tile([128, nsl], f32)
            nc.scalar.activation(junk2[:], parts[:, 0:nsl], AF.Copy,
                                 accum_out=sums[:, 0:1])
            nc.scalar.activation(junk2[:], parts[:, nsl:2 * nsl], AF.Copy,
                                 accum_out=sums[:, 1:2])
            ar_in = dram.tile([128, 2], f32)
            ar_out = dram.tile([128, 2], f32)
            nc.sync.dma_start(ar_in[:], sums[:])
            nc.gpsimd.collective_compute(
                "AllReduce", AO.add, replica_groups=[list(range(NCORES))],
                ins=[ar_in.opt()], outs=[ar_out.opt()])
            g4 = cpool.tile([32, 4, 2], f32)
            nc.sync.dma_start(
                g4[:], ar_out[:].rearrange("(g p) k -> p g k", g=4))
            t1 = cpool.tile([32, 2], f32)
            t2 = cpool.tile([32, 2], f32)
            tot = cpool.tile([32, 2], f32)
            nc.vector.tensor_tensor(t1[:], g4[:, 0, :], g4[:, 1, :], AO.add)
            nc.vector.tensor_tensor(t2[:], g4[:, 2, :], g4[:, 3, :], AO.add)
            nc.vector.tensor_tensor(tot[:], t1[:], t2[:], AO.add)
            mu = cpool.tile([32, 1], f32)
            nc.vector.tensor_scalar(mu[:], tot[:, 0:1], inv_E, None, op0=AO.mult)
            ms = cpool.tile([32, 1], f32)
            nc.vector.tensor_scalar(ms[:], tot[:, 1:2], inv_E, None, op0=AO.mult)
            var = cpool.tile([32, 1], f32)
            mu2 = cpool.tile([32, 1], f32)
            nc.vector.tensor_tensor(mu2[:], mu[:], mu[:], AO.mult)
            nc.vector.tensor_tensor(var[:], ms[:], mu2[:], AO.subtract)
            epsb = cpool.tile([32, 1], f32)
            nc.vector.memset(epsb[:], EPS)
            std = cpool.tile([32, 1], f32)
            nc.scalar.activation(std[:], var[:], AF.Sqrt, bias=epsb[:])
            rstd = cpool.tile([32, 1], f32)
            nc.vector.reciprocal(rstd[:], std[:])
            gam_t = cpool.tile([32, 1], f32)
            nc.sync.dma_start(gam_t[:], gamma[:])
            bet_t = cpool.tile([32, 1], f32)
            nc.sync.dma_start(bet_t[:], beta[:])
            a_t = cpool.tile([32, 1], f32)
            nc.vector.tensor_tensor(a_t[:], gam_t[:], rstd[:], AO.mult)
            nma = cpool.tile([32, 1], f32)
            nc.vector.scalar_tensor_tensor(nma[:], mu[:], -1.0, a_t[:],
                                           op0=AO.mult, op1=AO.mult)
            c_t = cpool.tile([32, 1], f32)
            nc.vector.tensor_tensor(c_t[:], bet_t[:], nma[:], AO.add)

            wi_t = cpool.tile([32, OUT], f32)
            nc.sync.dma_start(wi_t[:], W_i[:])
            wif = cpool.tile([32, OUT], f32)
            nc.vector.tensor_scalar(wif[:], wi_t[:], a_t[:], None, op0=AO.mult)
            bi_t = cpool.tile([OUT, 1], f32)
            nc.sync.dma_start(bi_t[:], b_i[:])
            pb0 = psum.tile([128, 512], f32, tag="pfin")
            nc.tensor.matmul(pb0[:OUT, 0:1], wif[:], c_t[:], start=True,
                             stop=True)
            bcol = cpool.tile([OUT, 1], f32)
            nc.vector.tensor_tensor(bcol[:], pb0[:OUT, 0:1], bi_t[:], AO.add)
            scr = dram.tile([OUT, 1], f32)
            nc.sync.dma_start(scr[:], bcol[:])
            # wiaug padded to 128 cols so fee matmuls define full psum banks
            wiaug = cpool.tile([33, 128], bf16)
            nc.vector.memset(wiaug[:], 0.0)
            nc.vector.tensor_copy(wiaug[0:32, :OUT], wif[:])
            nc.gpsimd.dma_start(wiaug[32:33, :OUT],
                                scr[:].rearrange("a b -> b a"))

            # ---- DRAM intermediates ----
            # base, partition-contiguous: col b*128+f on partition p holds
            # base[slot b*128+p, f]
            baseH2 = dram.tile([128, ES], bf16, name="baseH2")
            tlgs = ([] if USE_RDMA else
                    [dram.tile([plan.NROW, 128], bf16, name=f"tlgs{k}",
                               tag=f"tlgs{k}") for k in range(DEPTH)])
            tlgd = [dram.tile([NLOCP, 128], bf16, name=f"tlgd{k}",
                              tag=f"tlgd{k}") for k in range(DEPTH)]
            tggs = [dram.tile([plan.NGLOBR, 128], bf16, name=f"tggs{k}",
                              tag=f"tggs{k}") for k in range(DEPTH)]

            NWP, NROW = plan.NWP, plan.NROW
            RQ = NWP // 4
            if USE_RDMA:
                RD = [(0, k) for k in range(NCORES)]
                rsem_d = nc.alloc_semaphore("rsem_d")
                rsem_f = nc.alloc_semaphore("rsem_f")
                lsem_d = nc.alloc_semaphore("lsem_d")
                lsem_f = nc.alloc_semaphore("lsem_f")
                csem = nc.alloc_semaphore("csem")
                for _sm in (rsem_d, rsem_f, lsem_d, lsem_f, csem):
                    nc.gpsimd.sem_clear(_sm)
                pid_reg = nc.gpsimd.partition_id()
                sendbuf = cpool.tile([128, NWP, 128], bf16)
                recvbuf = cpool.tile([128, NCORES, RQ * 128], bf16)

            def exchange(e):
                """AllGather tables e: 4 rounds of XOR-relative broadcast
                through SBUF, each copied out to tggs[e] in DRAM."""
                if not USE_RDMA:
                    nc.gpsimd.collective_compute(
                        "AllGather", AO.bypass,
                        replica_groups=[list(range(NCORES))],
                        ins=[tlgs[e].opt()], outs=[tggs[e].opt()])
                    return
                dstv = tggs[e][:].rearrange("(r b p) f -> p r b f",
                                            p=128, b=NWP)
                srcv = recvbuf[:].rearrange("p r (b f) -> p r b f", f=128)
                for q in range(4):
                    g = e * 4 + q
                    if g > 0:
                        nc.gpsimd.wait_ge(rsem_f, 16 * g)
                    nc.gpsimd.remote_dma_broadcast(
                        recvbuf[:, bass.ts(pid_reg, 1), :],
                        sendbuf[:, q * RQ:(q + 1) * RQ, :],
                        remote_sem=rsem_d, local_sem=lsem_d, rdests=RD)
                    nc.gpsimd.trigger_dma(count=None)
                    nc.sync.wait_ge(rsem_d, 16 * (g + 1))
                    cp = nc.sync.dma_start(
                        dstv[:, :, q * RQ:(q + 1) * RQ, :],
                        srcv[:, :, :RQ, :])
                    cp.then_inc(csem, 16)
                    nc.gpsimd.wait_ge(csem, 16 * (g + 1))
                    nc.gpsimd.remote_sem_update_broadcast(
                        rsem_f, lsem_f, rdests=RD)
                    nc.gpsimd.trigger_dma(count=None)

            def build_S(s):
                """One-hot scatter matrices for superwindow s (bulk build)."""
                cap = plan.sw_cap[s]
                nblk = cap // 128
                o0 = int(plan.sw_off[s])
                maxnblk = max(plan.sw_cap) // 128
                sohc = pool.tile([128, maxnblk], f32, tag="sohc")
                nc.sync.dma_start(sohc[:, :nblk],
                                  sohb[:, o0 // 128:o0 // 128 + nblk])
                S = pool.tile([128, nblk, 128], bf16, tag="S")
                for b in range(nblk):
                    nc.vector.tensor_scalar(S[:, b, :], iota_b[:],
                                            sohc[:, b:b + 1], None,
                                            op0=AO.is_equal)
                return S

            def window_chunks(s, w):
                """(block) list of window w within superwindow s."""
                o = int(plan.sw_off[s])
                b0 = (int(plan.slotA[w]) - o) // 128
                nb = (int(plan.capA[w]) + int(plan.capB[w])) // 128
                return [b0 + c for c in range(nb)]

            def scatter_sw(s, S, src_tile):
                """One-hot scatter of src_tile rows into per-window psum."""
                pw = psum.tile([128, 512], f32, tag="pw")
                for wl, w in enumerate(plan.sw_windows[s]):
                    chunks = window_chunks(s, w)
                    for ci, b in enumerate(chunks):
                        nc.tensor.matmul(
                            pw[:, wl * 128:wl * 128 + OUT], S[:, b, :],
                            src_tile[:, b, :OUT], start=(ci == 0),
                            stop=(ci == len(chunks) - 1))
                return pw

            def finalize_sw(it, s, pw):
                """pw[:, wl*128:...] holds the scatter sums per window."""
                ws = plan.sw_windows[s]
                nw = len(ws)
                w0 = ws[0]
                hC = pool.tile([128, GW, 128], f32, tag="hC", name="hC")
                ttabs = (pool.tile([128, GW, 256], bf16, tag="ttabs",
                                   name="ttabs")
                         if it < DEPTH else None)
                for wl, w in enumerate(ws):
                    pwv = pw[:, wl * 128:wl * 128 + OUT]
                    nc.vector.tensor_scalar(hC[:, wl, :OUT], pwv,
                                            invdeg_t[:, w:w + 1], None,
                                            op0=AO.mult)
                    if it < DEPTH:
                        pf = psum.tile([128, 512], f32, tag="pfin")
                        nc.tensor.transpose(pf[:OUT, :128], hC[:, wl, :OUT],
                                            ident[:])
                        hT = pool.tile([OUT + 1, 128], bf16, tag="hT")
                        nc.vector.memset(hT[:], 1.0)
                        nc.vector.tensor_copy(hT[:OUT, :], pf[:OUT, :128])
                        nc.tensor.matmul(pf[:, 128:384], hT[:], whsd_t[:],
                                         start=True, stop=True)
                        nc.scalar.copy(ttabs[:, wl, :], pf[:, 128:384])
                rows = slice(w0 * 128, (w0 + nw) * 128)
                if it == 0:
                    nc.sync.dma_start(
                        out_fn[rows, :].rearrange("(b p) f -> p b f", p=128),
                        hC[:, :nw, :OUT])
                if it == DEPTH:
                    nc.sync.dma_start(
                        out_h[rows, :].rearrange("(b p) f -> p b f", p=128),
                        hC[:, :nw, :OUT])
                    return
                if USE_RDMA:
                    nc.vector.tensor_copy(sendbuf[:, w0:w0 + nw, :],
                                          ttabs[:, :nw, 0:128])
                else:
                    nc.sync.dma_start(
                        tlgs[it][rows, :].rearrange("(b p) f -> p b f", p=128),
                        ttabs[:, :nw, 0:128])
                nc.sync.dma_start(
                    tlgd[it][rows, :].rearrange("(b p) f -> p b f", p=128),
                    ttabs[:, :nw, 128:256])

            # ---- pre-pass + iter 0 ----
            for s in range(NSW):
                cap = plan.sw_cap[s]
                nblk = cap // 128
                o0 = int(plan.sw_off[s])
                efm_t = pool.tile([34, cap], bf16, tag="big0")
                nc.sync.dma_start(efm_t[:], efm[:, o0:o0 + cap])
                S = build_S(s)
                feT = pool.tile([OUT, cap], bf16, tag="big1")

                for gi, g0 in enumerate(range(0, cap, 512)):
                    g1 = min(g0 + 512, cap)
                    p1 = psum.tile([128, 512], f32, tag="pA")
                    nc.tensor.matmul(p1[:OUT, :g1 - g0], wiaug[:, :OUT],
                                     efm_t[0:33, g0:g1], start=True, stop=True)
                    if gi % 2 == 0:
                        nc.scalar.activation(feT[:, g0:g1], p1[:OUT, :g1 - g0],
                                             AF.Relu)
                    else:
                        nc.vector.tensor_scalar(feT[:, g0:g1],
                                                p1[:OUT, :g1 - g0], 0.0, None,
                                                op0=AO.max)

                # per-window chunk bookkeeping for interleaved scatter
                block2w = {}
                wlen, wdone = {}, {}
                for wl, w in enumerate(plan.sw_windows[s]):
                    chunks = window_chunks(s, w)
                    wlen[w] = len(chunks)
                    wdone[w] = 0
                    for b in chunks:
                        block2w[b] = (wl, w)
                pw = psum.tile([128, 512], f32, tag="pw")

                for gi, g0 in enumerate(range(0, cap, 512)):
                    g1 = min(g0 + 512, cap)
                    nb = (g1 - g0) // 128
                    pb = psum.tile([128, 512], f32, tag="pbase")
                    pf = psum.tile([128, 512], f32, tag="pA")
                    for ci in range(nb):
                        sl = slice(g0 + ci * 128, g0 + (ci + 1) * 128)
                        cl = slice(ci * 128, (ci + 1) * 128)
                        nc.tensor.matmul(pb[:, cl], feT[:, sl], whmid_t[:],
                                         start=True, stop=False)
                        nc.tensor.matmul(pb[:, cl], efm_t[32:34, sl],
                                         wp2_t[32:34, :], start=False,
                                         stop=True)
                        nc.tensor.matmul(pf[:, cl], efm_t[0:33, sl], wiaug[:],
                                         start=True, stop=True)
                    basec = pool.tile([128, 512], bf16, tag="basec", bufs=3)
                    fee = pool.tile([128, 4, 128], bf16, tag="fee", bufs=3)
                    fv = fee[:].rearrange("p b f -> p (b f)")
                    if gi % 2 == 0:
                        nc.vector.tensor_copy(basec[:, :g1 - g0],
                                              pb[:, :g1 - g0])
                        nc.scalar.activation(fv[:, :g1 - g0], pf[:, :g1 - g0],
                                             AF.Relu)
                    else:
                        nc.scalar.copy(basec[:, :g1 - g0], pb[:, :g1 - g0])
                        nc.vector.tensor_scalar(fv[:, :g1 - g0],
                                                pf[:, :g1 - g0],
                                                0.0, None, op0=AO.max)
                    nc.sync.dma_start(baseH2[:, o0 + g0:o0 + g1],
                                      basec[:, :g1 - g0])
                    for ci in range(nb):
                        b = g0 // 128 + ci
                        wl, w = block2w[b]
                        k = wdone[w]
                        wdone[w] = k + 1
                        nc.tensor.matmul(
                            pw[:, wl * 128:wl * 128 + OUT], S[:, b, :],
                            fee[:, ci, :OUT], start=(k == 0),
                            stop=(k == wlen[w] - 1))
                finalize_sw(0, s, pw)

            exchange(0)

            # ---- iterations 1..DEPTH ----
            for it in range(1, DEPTH + 1):
                if USE_RDMA and it < DEPTH:
                    # sendbuf rewritten this iteration: prior sends must be out
                    nc.vector.wait_ge(lsem_d, 64 * it)
                for s in range(NSW):
                    cap = plan.sw_cap[s]
                    nblk = cap // 128
                    capA = plan.sw_capA[s]
                    capB = plan.sw_capB[s]
                    o0 = int(plan.sw_off[s])
                    GS = pool.tile([128, nblk, 128], bf16, tag="big0")
                    GD = pool.tile([128, nblk, 128], bf16, tag="big1")
                    BASE = pool.tile([128, nblk, 128], bf16, tag="big2")
                    nc.sync.dma_start(
                        BASE[:].rearrange("p b f -> p (b f)"),
                        baseH2[:, o0:o0 + cap])
                    S = build_S(s)
                    gsix = pool.tile([128, cap // 16], i16, tag="gsix")
                    nc.sync.dma_start(gsix[:],
                                      gs_idx[:, o0 // 16:(o0 + cap) // 16])
                    gdix = pool.tile([128, cap // 16], i16, tag="gdix")
                    nc.sync.dma_start(gdix[:],
                                      gd_idx[:, o0 // 16:(o0 + cap) // 16])

                    GPC = 1024   # gather piece size (fits the SWDGE ring)
                    for q0 in range(0, cap, GPC):
                        m = min(GPC, cap - q0)
                        nc.gpsimd.dma_gather(
                            GD[:, q0 // 128:(q0 + m) // 128, :],
                            tlgd[it - 1][:], gdix[:, q0 // 16:(q0 + m) // 16],
                            m, m, 128, elem_step=128)
                    for w in plan.sw_windows[s]:
                        regs = ((int(plan.slotA[w]) - o0, int(plan.capA[w]),
                                 tggs[it - 1][:]),
                                (int(plan.slotB[w]) - o0, int(plan.capB[w]),
                                 tggs[it - 1][SPLIT:]))
                        for r0, ln, srcv in regs:
                            for q0 in range(r0, r0 + ln, GPC):
                                m = min(GPC, r0 + ln - q0)
                                nc.gpsimd.dma_gather(
                                    GS[:, q0 // 128:(q0 + m) // 128, :], srcv,
                                    gsix[:, q0 // 16:(q0 + m) // 16],
                                    m, m, 128, elem_step=128)

                    nc.vector.tensor_tensor(GS[:], GS[:], BASE[:], AO.add)
                    nc.vector.tensor_tensor(GS[:], GS[:], GD[:], AO.add)
                    nc.scalar.activation(GS[:], GS[:], AF.Relu)

                    pw = scatter_sw(s, S, GS)
                    finalize_sw(it, s, pw)
                if it < DEPTH:
                    exchange(it)

    nc.compile()
    return nc


_CACHE = {}


def kernel(e, p, gamma, beta, W_i, b_i, W_h, b_h, src, dst, num_nodes):
    e = np.asarray(e, np.float32)
    p = np.asarray(p, np.float32)
    src = np.asarray(src, np.int64)
    dst = np.asarray(dst, np.int64)
    N = int(num_nodes)
    OUT = int(np.asarray(W_i).shape[1])

    plan = Plan(src, dst, N)
    sig = plan.signature()
    if sig not in _CACHE:
        _CACHE[sig] = _build(plan, OUT)
    nc = _CACHE[sig]

    per_core = _host_inputs(plan, e, p, src, dst)
    wts = _weight_inputs(plan, np.asarray(gamma), np.asarray(beta),
                         np.asarray(W_i), np.asarray(b_i),
                         np.asarray(W_h), np.asarray(b_h))
    in_maps = [dict(m, **wts) for m in per_core]

    res = run_bass_kernel_spmd(nc, in_maps, core_ids=list(range(NCORES)))
    fn = np.concatenate([np.asarray(res.results[r]["out_fn"],
                                    np.float32)[:plan.NLOC]
                         for r in range(NCORES)], 0)[:N]
    h = np.concatenate([np.asarray(res.results[r]["out_h"],
                                   np.float32)[:plan.NLOC]
                        for r in range(NCORES)], 0)[:N]
    return np.concatenate([fn, h], axis=1)


# revision 33
# speedup vs baseline: 1.0091x; 1.0091x over previous
"""GCN encoder (edge-wise message passing) on 8 Trainium2 NeuronCores.

Strategy (dst-range sharding):
  - Host: sort edges by dst, shard by dst-range (core r owns nodes
    [r*NLOC, (r+1)*NLOC)), group edges into 128-node windows, pad each
    (window, src-half) group to 128-multiples. Degree / index prep on host.
  - Device: BN stats via ACT-accumulate + tiny AllReduce, folded into W_i.
    Pre-pass computes f_e (feature-major) once, materializes the
    loop-invariant per-edge base = f_e @ Wh_mid + p*w_p to HBM bf16 in a
    partition-contiguous layout, and performs the iter-0 scatter.
    Each iteration: batched per-superwindow dma_gathers pull g_s[src] rows
    from the AllGathered global src-table and g_d[dst] rows from the local
    dst-table; eh = relu(base + g_s + g_d) via two in-place DVE adds + one
    ACT relu; scatter-mean via one-hot matmul into PSUM per 128-node
    window (the one-hot S matrices are built in bulk with stride-0
    broadcast is_equal ops, split across DVE and GPSIMD); then next tables
    g_s|g_d = h @ [Wh_src|Wh_dst] (+b_h), src half AllGathered.
"""
import sys
sys.path.insert(0, "/opt/trn_rl_repo")

import numpy as np
import ml_dtypes
from contextlib import ExitStack

from concourse import bass, bacc, mybir, tile, masks
from concourse.bass_utils import run_bass_kernel_spmd

f32 = mybir.dt.float32
bf16 = mybir.dt.bfloat16
i16 = mybir.dt.int16
i32 = mybir.dt.int32
AO = mybir.AluOpType
AF = mybir.ActivationFunctionType

NCORES = 8
DEPTH = 3
EPS = 1e-5
GW = 4            # windows per superwindow
USE_RDMA = False   # replace table AllGather with remote_dma_broadcast rounds
STAT_SLICE = 1024
S_DVE_BLOCKS = 38   # per-sw S-build blocks on DVE (rest on GPSIMD), /64ths

bfl = ml_dtypes.bfloat16


def _ru(x, m):
    return (x + m - 1) // m * m


class Plan:
    """Host-side preprocessing: sharding, sorting, padding, index layout."""

    def __init__(self, src, dst, N):
        E = src.shape[0]
        self.N, self.E = N, E
        self.NLOC = (N + NCORES - 1) // NCORES
        self.NWIN = (self.NLOC + 127) // 128
        self.NLOCP = self.NWIN * 128
        self.NGLOB = NCORES * self.NLOCP
        # rdma allgather: pad per-core table rows to 4 equal rounds
        self.NWP = _ru(self.NWIN, 4)
        self.NROW = self.NWP * 128
        self.NGLOBR = NCORES * self.NROW
        # src-half split: largest rank-multiple of NROW that fits int16
        self.SPLIT = min((32768 // self.NROW) * self.NROW, self.NGLOBR)
        assert self.NGLOBR - self.SPLIT < 32768

        owner = dst // self.NLOC
        local = dst - owner * self.NLOC
        win = local >> 7
        self.ohval_all = (local & 127).astype(np.float32)
        srcrow = (src // self.NLOC) * self.NROW + (src % self.NLOC)
        half = (srcrow >= self.SPLIT).astype(np.int64)
        self.srcrow, self.local, self.owner, self.win, self.half = (
            srcrow, local, owner, win, half)

        key = (owner * self.NWIN + win) * 2 + half
        self.order = np.argsort(key, kind="stable")
        cnt = np.bincount(key, minlength=NCORES * self.NWIN * 2)
        cnt = cnt.reshape(NCORES, self.NWIN, 2)
        self.capA = np.maximum(_ru(cnt[:, :, 0].max(0), 128), 128)
        self.capB = _ru(cnt[:, :, 1].max(0), 128)
        self.cnt = cnt

        # superwindows
        self.NSW = (self.NWIN + GW - 1) // GW
        self.sw_windows = [list(range(s * GW, min((s + 1) * GW, self.NWIN)))
                           for s in range(self.NSW)]
        # slot layout: per sw, [A_w0 B_w0 | A_w1 B_w1 | ...] so each
        # window's chunks are contiguous (single open psum group at a time)
        self.slotA = np.zeros(self.NWIN, np.int64)   # slot offset of A group
        self.slotB = np.zeros(self.NWIN, np.int64)
        self.sw_off = np.zeros(self.NSW + 1, np.int64)
        off = 0
        for s, ws in enumerate(self.sw_windows):
            self.sw_off[s] = off
            a = off
            for w in ws:
                self.slotA[w] = a
                a += self.capA[w]
                self.slotB[w] = a
                a += self.capB[w]
            off = a
        self.sw_off[self.NSW] = off
        self.ES = int(off)
        self.sw_capA = [int(sum(self.capA[w] for w in ws))
                        for ws in self.sw_windows]
        self.sw_capB = [int(sum(self.capB[w] for w in ws))
                        for ws in self.sw_windows]
        self.sw_cap = [a + b for a, b in zip(self.sw_capA, self.sw_capB)]
        self.EMAX4 = _ru(max(int((owner == r).sum()) for r in range(NCORES)), 512)
        self.Q4 = self.EMAX4 // 4

    def signature(self):
        return (self.N, self.E, tuple(self.capA), tuple(self.capB))


def _host_inputs(plan, e, p, src, dst):
    """Build the per-core input arrays."""
    NLOC, NWIN, ES = plan.NLOC, plan.NWIN, plan.ES
    order, cnt = plan.order, plan.cnt
    deg = np.maximum(np.bincount(dst, minlength=plan.N), 1).astype(np.float32)
    invd = 1.0 / deg

    in_maps = []
    pos = 0
    # order slices per (r, w, h) in key order
    slices = {}
    for r in range(NCORES):
        for w in range(NWIN):
            for h in range(2):
                c = int(cnt[r, w, h])
                slices[(r, w, h)] = order[pos:pos + c]
                pos += c
    assert pos == plan.E

    for r in range(NCORES):
        efm = np.zeros((34, ES), np.float32)
        efm[32, :] = 1.0
        gsx = np.zeros(ES, np.int16)
        gdx = np.zeros(ES, np.int16)
        ohv = np.full(ES, -5.0, np.float32)
        for w in range(NWIN):
            for h, base_slot in ((0, plan.slotA[w]), (1, plan.slotB[w])):
                idx = slices[(r, w, h)]
                n = idx.shape[0]
                sl = slice(base_slot, base_slot + n)
                efm[0:32, sl] = e[idx].T
                efm[33, sl] = p[idx, 0]
                gsx[sl] = plan.srcrow[idx] - (plan.SPLIT if h else 0)
                gdx[sl] = plan.local[idx]
                ohv[sl] = plan.ohval_all[idx]

        # wrap idxs per superwindow: [16, cap/16] replicated x8
        def wrap(arr):
            out = np.zeros((128, ES // 16), np.int16)
            for s in range(plan.NSW):
                o0, o1 = int(plan.sw_off[s]), int(plan.sw_off[s + 1])
                seg = arr[o0:o1].reshape(-1, 16).T
                out[:, o0 // 16:o1 // 16] = np.tile(seg, (8, 1))
            return out

        soh = ohv.reshape(-1, 128).T.copy()  # [128, ES//128]
        ivl = np.ones(plan.NLOCP, np.float32)
        lo, hi = r * NLOC, min((r + 1) * NLOC, plan.N)
        ivl[:hi - lo] = invd[lo:hi]
        invdeg = ivl.reshape(NWIN, 128).T.copy()  # [128, NWIN]

        mask = plan.owner == np.int64(r)
        er = e[mask]
        epad = np.zeros((plan.EMAX4, 32), np.float32)
        epad[:er.shape[0]] = er
        e4 = epad.reshape(4, plan.Q4, 32).transpose(0, 2, 1).reshape(128, plan.Q4)

        in_maps.append({
            "efm": efm.astype(bfl),
            "gs_idx": wrap(gsx),
            "gd_idx": wrap(gdx),
            "sohb": soh,
            "invdeg": invdeg,
            "e4": e4.astype(bfl),
        })
    return in_maps


def _weight_inputs(plan, gamma, beta, W_i, b_i, W_h, b_h):
    OUT = W_i.shape[1]
    whmid = np.zeros((OUT, 128), np.float32)
    whmid[:, :OUT] = W_h[OUT:2 * OUT]
    wp2 = np.zeros((2, 128), np.float32)
    wp2[1, :OUT] = W_h[2 * OUT]
    whsd = np.zeros((OUT + 1, 256), np.float32)
    whsd[:OUT, 0:OUT] = W_h[0:OUT]
    whsd[:OUT, 128:128 + OUT] = W_h[2 * OUT + 1:3 * OUT + 1]
    whsd[OUT, 128:128 + OUT] = b_h
    return {
        "W_i": W_i.astype(np.float32),
        "b_i": b_i.reshape(OUT, 1).astype(np.float32),
        "gamma": gamma.reshape(32, 1).astype(np.float32),
        "beta": beta.reshape(32, 1).astype(np.float32),
        "whmid": whmid.astype(bfl),
        "wp2": wp2.astype(bfl),
        "whsd": whsd.astype(bfl),
    }


def _build(plan, OUT):
    """Build + compile the SPMD Bass program for this plan."""
    NWIN, NSW, ES = plan.NWIN, plan.NSW, plan.ES
    NLOCP, NGLOB, SPLIT = plan.NLOCP, plan.NGLOB, plan.SPLIT
    IN = 32

    nc = bacc.Bacc("TRN2", target_bir_lowering=False, debug=False,
                   num_devices=NCORES)

    efm = nc.dram_tensor("efm", [34, ES], bf16, kind="ExternalInput")
    gs_idx = nc.dram_tensor("gs_idx", [128, ES // 16], i16, kind="ExternalInput")
    gd_idx = nc.dram_tensor("gd_idx", [128, ES // 16], i16, kind="ExternalInput")
    sohb = nc.dram_tensor("sohb", [128, ES // 128], f32, kind="ExternalInput")
    invdeg = nc.dram_tensor("invdeg", [128, NWIN], f32, kind="ExternalInput")
    e4 = nc.dram_tensor("e4", [128, plan.Q4], bf16, kind="ExternalInput")
    W_i = nc.dram_tensor("W_i", [IN, OUT], f32, kind="ExternalInput")
    b_i = nc.dram_tensor("b_i", [OUT, 1], f32, kind="ExternalInput")
    gamma = nc.dram_tensor("gamma", [IN, 1], f32, kind="ExternalInput")
    beta = nc.dram_tensor("beta", [IN, 1], f32, kind="ExternalInput")
    whmid = nc.dram_tensor("whmid", [OUT, 128], bf16, kind="ExternalInput")
    wp2 = nc.dram_tensor("wp2", [2, 128], bf16, kind="ExternalInput")
    whsd = nc.dram_tensor("whsd", [OUT + 1, 256], bf16, kind="ExternalInput")

    out_fn = nc.dram_tensor("out_fn", [NLOCP, OUT], f32, kind="ExternalOutput")
    out_h = nc.dram_tensor("out_h", [NLOCP, OUT], f32, kind="ExternalOutput")

    inv_E = 1.0 / plan.E

    with tile.TileContext(nc) as tc:
        with ExitStack() as ctx:
            cpool = ctx.enter_context(tc.tile_pool(name="cpool", bufs=1))
            pool = ctx.enter_context(tc.tile_pool(name="pool", bufs=2))
            spool = ctx.enter_context(tc.tile_pool(name="spool", bufs=2))
            psum = ctx.enter_context(tc.tile_pool(name="psum", bufs=2,
                                                  space="PSUM"))
            dram = ctx.enter_context(tc.tile_pool(name="dram", bufs=1,
                                                  space="DRAM"))

            # ---- constants ----
            iota_i = cpool.tile([128, 128], i32)
            nc.gpsimd.iota(iota_i[:], pattern=[[1, 128]], base=0,
                           channel_multiplier=0)
            iota_b = cpool.tile([128, 128], bf16)
            nc.vector.tensor_copy(iota_b[:], iota_i[:])
            ident = cpool.tile([128, 128], f32)
            masks.make_identity(nc, ident[:])

            whmid_t = cpool.tile([OUT, 128], bf16)
            nc.sync.dma_start(whmid_t[:], whmid[:])
            wp2_t = cpool.tile([34, 128], bf16)
            nc.sync.dma_start(wp2_t[32:34, :], wp2[:])
            whsd_t = cpool.tile([OUT + 1, 256], bf16)
            nc.sync.dma_start(whsd_t[:], whsd[:])
            invdeg_t = cpool.tile([128, NWIN], f32)
            nc.sync.dma_start(invdeg_t[:], invdeg[:])

            # ---- BN stats: per-core partial sums of e, e^2 ----
            nsl = (plan.Q4 + STAT_SLICE - 1) // STAT_SLICE
            parts = cpool.tile([128, 2 * nsl], f32)
            for s in range(nsl):
                c0, c1 = s * STAT_SLICE, min((s + 1) * STAT_SLICE, plan.Q4)
                esl = spool.tile([128, STAT_SLICE], bf16, tag="esl")
                nc.sync.dma_start(esl[:, :c1 - c0], e4[:, c0:c1])
                junk = spool.tile([128, STAT_SLICE], bf16, tag="junk")
                nc.scalar.activation(junk[:, :c1 - c0], esl[:, :c1 - c0],
                                     AF.Copy, accum_out=parts[:, s:s + 1])
                nc.scalar.activation(junk[:, :c1 - c0], esl[:, :c1 - c0],
                                     AF.Square,
                                     accum_out=parts[:, nsl + s:nsl + s + 1])
            sums = cpool.tile([128, 2], f32)
            junk2 = cpool.tile([128, nsl], f32)
            nc.scalar.activation(junk2[:], parts[:, 0:nsl], AF.Copy,
                                 accum_out=sums[:, 0:1])
            nc.scalar.activation(junk2[:], parts[:, nsl:2 * nsl], AF.Copy,
                                 accum_out=sums[:, 1:2])
            ar_in = dram.tile([128, 2], f32)
            ar_out = dram.tile([128, 2], f32)
            nc.sync.dma_start(ar_in[:], sums[:])
            nc.gpsimd.collective_compute(
                "AllReduce", AO.add, replica_groups=[list(range(NCORES))],
                ins=[ar_in.opt()], outs=[ar_out.opt()])
            g4 = cpool.tile([32, 4, 2], f32)
            nc.sync.dma_start(
                g4[:], ar_out[:].rearrange("(g p) k -> p g k", g=4))
            t1 = cpool.tile([32, 2], f32)
            t2 = cpool.tile([32, 2], f32)
            tot = cpool.tile([32, 2], f32)
            nc.vector.tensor_tensor(t1[:], g4[:, 0, :], g4[:, 1, :], AO.add)
            nc.vector.tensor_tensor(t2[:], g4[:, 2, :], g4[:, 3, :], AO.add)
            nc.vector.tensor_tensor(tot[:], t1[:], t2[:], AO.add)
            mu = cpool.tile([32, 1], f32)
            nc.vector.tensor_scalar(mu[:], tot[:, 0:1], inv_E, None, op0=AO.mult)
            ms = cpool.tile([32, 1], f32)
            nc.vector.tensor_scalar(ms[:], tot[:, 1:2], inv_E, None, op0=AO.mult)
            var = cpool.tile([32, 1], f32)
            mu2 = cpool.tile([32, 1], f32)
            nc.vector.tensor_tensor(mu2[:], mu[:], mu[:], AO.mult)
            nc.vector.tensor_tensor(var[:], ms[:], mu2[:], AO.subtract)
            epsb = cpool.tile([32, 1], f32)
            nc.vector.memset(epsb[:], EPS)
            std = cpool.tile([32, 1], f32)
            nc.scalar.activation(std[:], var[:], AF.Sqrt, bias=epsb[:])
            rstd = cpool.tile([32, 1], f32)
            nc.vector.reciprocal(rstd[:], std[:])
            gam_t = cpool.tile([32, 1], f32)
            nc.sync.dma_start(gam_t[:], gamma[:])
            bet_t = cpool.tile([32, 1], f32)
            nc.sync.dma_start(bet_t[:], beta[:])
            a_t = cpool.tile([32, 1], f32)
            nc.vector.tensor_tensor(a_t[:], gam_t[:], rstd[:], AO.mult)
            nma = cpool.tile([32, 1], f32)
            nc.vector.scalar_tensor_tensor(nma[:], mu[:], -1.0, a_t[:],
                                           op0=AO.mult, op1=AO.mult)
            c_t = cpool.tile([32, 1], f32)
            nc.vector.tensor_tensor(c_t[:], bet_t[:], nma[:], AO.add)

            wi_t = cpool.tile([32, OUT], f32)
            nc.sync.dma_start(wi_t[:], W_i[:])
            wif = cpool.tile([32, OUT], f32)
            nc.vector.tensor_scalar(wif[:], wi_t[:], a_t[:], None, op0=AO.mult)
            bi_t = cpool.tile([OUT, 1], f32)
            nc.sync.dma_start(bi_t[:], b_i[:])
            pb0 = psum.tile([128, 512], f32, tag="pfin")
            nc.tensor.matmul(pb0[:OUT, 0:1], wif[:], c_t[:], start=True,
                             stop=True)
            bcol = cpool.tile([OUT, 1], f32)
            nc.vector.tensor_tensor(bcol[:], pb0[:OUT, 0:1], bi_t[:], AO.add)
            scr = dram.tile([OUT, 1], f32)
            nc.sync.dma_start(scr[:], bcol[:])
            # wiaug padded to 128 cols so fee matmuls define full psum banks
            wiaug = cpool.tile([33, 128], bf16)
            nc.vector.memset(wiaug[:], 0.0)
            nc.vector.tensor_copy(wiaug[0:32, :OUT], wif[:])
            nc.gpsimd.dma_start(wiaug[32:33, :OUT],
                                scr[:].rearrange("a b -> b a"))

            # ---- DRAM intermediates ----
            # base, partition-contiguous: col b*128+f on partition p holds
            # base[slot b*128+p, f]
            baseH2 = dram.tile([128, ES], bf16, name="baseH2")
            tlgs = ([] if USE_RDMA else
                    [dram.tile([plan.NROW, 128], bf16, name=f"tlgs{k}",
                               tag=f"tlgs{k}") for k in range(DEPTH)])
            tlgd = [dram.tile([NLOCP, 128], bf16, name=f"tlgd{k}",
                              tag=f"tlgd{k}") for k in range(DEPTH)]
            tggs = [dram.tile([plan.NGLOBR, 128], bf16, name=f"tggs{k}",
                              tag=f"tggs{k}") for k in range(DEPTH)]

            NWP, NROW = plan.NWP, plan.NROW
            RQ = NWP // 4
            if USE_RDMA:
                RD = [(0, k) for k in range(NCORES)]
                rsem_d = nc.alloc_semaphore("rsem_d")
                rsem_f = nc.alloc_semaphore("rsem_f")
                lsem_d = nc.alloc_semaphore("lsem_d")
                lsem_f = nc.alloc_semaphore("lsem_f")
                csem = nc.alloc_semaphore("csem")
                for _sm in (rsem_d, rsem_f, lsem_d, lsem_f, csem):
                    nc.gpsimd.sem_clear(_sm)
                pid_reg = nc.gpsimd.partition_id()
                sendbuf = cpool.tile([128, NWP, 128], bf16)
                recvbuf = cpool.tile([128, NCORES, RQ * 128], bf16)

            def exchange(e):
                """AllGather tables e: 4 rounds of XOR-relative broadcast
                through SBUF, each copied out to tggs[e] in DRAM."""
                if not USE_RDMA:
                    nc.gpsimd.collective_compute(
                        "AllGather", AO.bypass,
                        replica_groups=[list(range(NCORES))],
                        ins=[tlgs[e].opt()], outs=[tggs[e].opt()])
                    return
                dstv = tggs[e][:].rearrange("(r b p) f -> p r b f",
                                            p=128, b=NWP)
                srcv = recvbuf[:].rearrange("p r (b f) -> p r b f", f=128)
                for q in range(4):
                    g = e * 4 + q
                    if g > 0:
                        nc.gpsimd.wait_ge(rsem_f, 16 * g)
                    nc.gpsimd.remote_dma_broadcast(
                        recvbuf[:, bass.ts(pid_reg, 1), :],
                        sendbuf[:, q * RQ:(q + 1) * RQ, :],
                        remote_sem=rsem_d, local_sem=lsem_d, rdests=RD)
                    nc.gpsimd.trigger_dma(count=None)
                    nc.sync.wait_ge(rsem_d, 16 * (g + 1))
                    cp = nc.sync.dma_start(
                        dstv[:, :, q * RQ:(q + 1) * RQ, :],
                        srcv[:, :, :RQ, :])
                    cp.then_inc(csem, 16)
                    nc.gpsimd.wait_ge(csem, 16 * (g + 1))
                    nc.gpsimd.remote_sem_update_broadcast(
                        rsem_f, lsem_f, rdests=RD)
                    nc.gpsimd.trigger_dma(count=None)

            def build_S(s):
                """One-hot scatter matrices for superwindow s (bulk build)."""
                cap = plan.sw_cap[s]
                nblk = cap // 128
                o0 = int(plan.sw_off[s])
                maxnblk = max(plan.sw_cap) // 128
                sohc = pool.tile([128, maxnblk], f32, tag="sohc")
                nc.sync.dma_start(sohc[:, :nblk],
                                  sohb[:, o0 // 128:o0 // 128 + nblk])
                S = pool.tile([128, nblk, 128], bf16, tag="S")
                for b in range(nblk):
                    nc.vector.tensor_scalar(S[:, b, :], iota_b[:],
                                            sohc[:, b:b + 1], None,
                                            op0=AO.is_equal)
                return S

            def window_chunks(s, w):
                """(block) list of window w within superwindow s."""
                o = int(plan.sw_off[s])
                b0 = (int(plan.slotA[w]) - o) // 128
                nb = (int(plan.capA[w]) + int(plan.capB[w])) // 128
                return [b0 + c for c in range(nb)]

            def scatter_sw(s, S, src_tile):
                """One-hot scatter of src_tile rows into per-window psum."""
                pw = psum.tile([128, 512], f32, tag="pw")
                for wl, w in enumerate(plan.sw_windows[s]):
                    chunks = window_chunks(s, w)
                    for ci, b in enumerate(chunks):
                        nc.tensor.matmul(
                            pw[:, wl * 128:wl * 128 + OUT], S[:, b, :],
                            src_tile[:, b, :OUT], start=(ci == 0),
                            stop=(ci == len(chunks) - 1))
                return pw

            def finalize_sw(it, s, pw):
                """pw[:, wl*128:...] holds the scatter sums per window."""
                ws = plan.sw_windows[s]
                nw = len(ws)
                w0 = ws[0]
                hC = pool.tile([128, GW, 128], f32, tag="hC", name="hC")
                ttabs = (pool.tile([128, GW, 256], bf16, tag="ttabs",
                                   name="ttabs")
                         if it < DEPTH else None)
                for wl, w in enumerate(ws):
                    pwv = pw[:, wl * 128:wl * 128 + OUT]
                    nc.vector.tensor_scalar(hC[:, wl, :OUT], pwv,
                                            invdeg_t[:, w:w + 1], None,
                                            op0=AO.mult)
                    if it < DEPTH:
                        pf = psum.tile([128, 512], f32, tag="pfin")
                        nc.tensor.transpose(pf[:OUT, :128], hC[:, wl, :OUT],
                                            ident[:])
                        hT = pool.tile([OUT + 1, 128], bf16, tag="hT")
                        nc.vector.memset(hT[:], 1.0)
                        nc.vector.tensor_copy(hT[:OUT, :], pf[:OUT, :128])
                        nc.tensor.matmul(pf[:, 128:384], hT[:], whsd_t[:],
                                         start=True, stop=True)
                        nc.scalar.copy(ttabs[:, wl, :], pf[:, 128:384])
                rows = slice(w0 * 128, (w0 + nw) * 128)
                if it == 0:
                    nc.sync.dma_start(
                        out_fn[rows, :].rearrange("(b p) f -> p b f", p=128),
                        hC[:, :nw, :OUT])
                if it == DEPTH:
                    nc.sync.dma_start(
                        out_h[rows, :].rearrange("(b p) f -> p b f", p=128),
                        hC[:, :nw, :OUT])
                    return
                if USE_RDMA:
                    nc.vector.tensor_copy(sendbuf[:, w0:w0 + nw, :],
                                          ttabs[:, :nw, 0:128])
                else:
                    nc.sync.dma_start(
                        tlgs[it][rows, :].rearrange("(b p) f -> p b f", p=128),
                        ttabs[:, :nw, 0:128])
                nc.sync.dma_start(
                    tlgd[it][rows, :].rearrange("(b p) f -> p b f", p=128),
                    ttabs[:, :nw, 128:256])

            # ---- pre-pass + iter 0 ----
            for s in range(NSW):
                cap = plan.sw_cap[s]
                nblk = cap // 128
                o0 = int(plan.sw_off[s])
                efm_t = pool.tile([34, cap], bf16, tag="big0")
                nc.sync.dma_start(efm_t[:], efm[:, o0:o0 + cap])
                S = build_S(s)
                feT = pool.tile([OUT, cap], bf16, tag="big1")

                for gi, g0 in enumerate(range(0, cap, 512)):
                    g1 = min(g0 + 512, cap)
                    p1 = psum.tile([128, 512], f32, tag="pA")
                    nc.tensor.matmul(p1[:OUT, :g1 - g0], wiaug[:, :OUT],
                                     efm_t[0:33, g0:g1], start=True, stop=True)
                    if gi % 2 == 0:
                        nc.scalar.activation(feT[:, g0:g1], p1[:OUT, :g1 - g0],
                                             AF.Relu)
                    else:
                        nc.vector.tensor_scalar(feT[:, g0:g1],
                                                p1[:OUT, :g1 - g0], 0.0, None,
                                                op0=AO.max)

                # per-window chunk bookkeeping for interleaved scatter
                block2w = {}
                wlen, wdone = {}, {}
                for wl, w in enumerate(plan.sw_windows[s]):
                    chunks = window_chunks(s, w)
                    wlen[w] = len(chunks)
                    wdone[w] = 0
                    for b in chunks:
                        block2w[b] = (wl, w)
                pw = psum.tile([128, 512], f32, tag="pw")

                for gi, g0 in enumerate(range(0, cap, 512)):
                    g1 = min(g0 + 512, cap)
                    nb = (g1 - g0) // 128
                    pb = psum.tile([128, 512], f32, tag="pbase")
                    pf = psum.tile([128, 512], f32, tag="pA")
                    for ci in range(nb):
                        sl = slice(g0 + ci * 128, g0 + (ci + 1) * 128)
                        cl = slice(ci * 128, (ci + 1) * 128)
                        nc.tensor.matmul(pb[:, cl], feT[:, sl], whmid_t[:],
                                         start=True, stop=False)
                        nc.tensor.matmul(pb[:, cl], efm_t[32:34, sl],
                                         wp2_t[32:34, :], start=False,
                                         stop=True)
                        nc.tensor.matmul(pf[:, cl], efm_t[0:33, sl], wiaug[:],
                                         start=True, stop=True)
                    basec = pool.tile([128, 512], bf16, tag="basec", bufs=3)
                    fee = pool.tile([128, 4, 128], bf16, tag="fee", bufs=3)
                    fv = fee[:].rearrange("p b f -> p (b f)")
                    if gi % 2 == 0:
                        nc.vector.tensor_copy(basec[:, :g1 - g0],
                                              pb[:, :g1 - g0])
                        nc.scalar.activation(fv[:, :g1 - g0], pf[:, :g1 - g0],
                                             AF.Relu)
                    else:
                        nc.scalar.copy(basec[:, :g1 - g0], pb[:, :g1 - g0])
                        nc.vector.tensor_scalar(fv[:, :g1 - g0],
                                                pf[:, :g1 - g0],
                                                0.0, None, op0=AO.max)
                    nc.sync.dma_start(baseH2[:, o0 + g0:o0 + g1],
                                      basec[:, :g1 - g0])
                    for ci in range(nb):
                        b = g0 // 128 + ci
                        wl, w = block2w[b]
                        k = wdone[w]
                        wdone[w] = k + 1
                        nc.tensor.matmul(
                            pw[:, wl * 128:wl * 128 + OUT], S[:, b, :],
                            fee[:, ci, :OUT], start=(k == 0),
                            stop=(k == wlen[w] - 1))
                finalize_sw(0, s, pw)

            exchange(0)

            # ---- iterations 1..DEPTH ----
            for it in range(1, DEPTH + 1):
                if USE_RDMA and it < DEPTH:
                    # sendbuf rewritten this iteration: prior sends must be out
                    nc.vector.wait_ge(lsem_d, 64 * it)
                for s in range(NSW):
                    cap = plan.sw_cap[s]
                    nblk = cap // 128
                    capA = plan.sw_capA[s]
                    capB = plan.sw_capB[s]
                    o0 = int(plan.sw_off[s])
                    GS = pool.tile([128, nblk, 128], bf16, tag="big0")
                    GD = pool.tile([128, nblk, 128], bf16, tag="big1")
                    BASE = pool.tile([128, nblk, 128], bf16, tag="big2")
                    nc.sync.dma_start(
                        BASE[:].rearrange("p b f -> p (b f)"),
                        baseH2[:, o0:o0 + cap])
                    S = build_S(s)
                    gsix = pool.tile([128, cap // 16], i16, tag="gsix")
                    nc.sync.dma_start(gsix[:],
                                      gs_idx[:, o0 // 16:(o0 + cap) // 16])
                    gdix = pool.tile([128, cap // 16], i16, tag="gdix")
                    nc.sync.dma_start(gdix[:],
                                      gd_idx[:, o0 // 16:(o0 + cap) // 16])

                    GPC = 1024   # gather piece size (fits the SWDGE ring)
                    for q0 in range(0, cap, GPC):
                        m = min(GPC, cap - q0)
                        nc.gpsimd.dma_gather(
                            GD[:, q0 // 128:(q0 + m) // 128, :],
                            tlgd[it - 1][:], gdix[:, q0 // 16:(q0 + m) // 16],
                            m, m, 128, elem_step=128)
                    for w in plan.sw_windows[s]:
                        regs = ((int(plan.slotA[w]) - o0, int(plan.capA[w]),
                                 tggs[it - 1][:]),
                                (int(plan.slotB[w]) - o0, int(plan.capB[w]),
                                 tggs[it - 1][SPLIT:]))
                        for r0, ln, srcv in regs:
                            for q0 in range(r0, r0 + ln, GPC):
                                m = min(GPC, r0 + ln - q0)
                                nc.gpsimd.dma_gather(
                                    GS[:, q0 // 128:(q0 + m) // 128, :], srcv,
                                    gsix[:, q0 // 16:(q0 + m) // 16],
                                    m, m, 128, elem_step=128)

                    qs = max(1, nblk // 4)
                    bnds = list(range(0, nblk, qs)) + [nblk]
                    for b0, b1 in zip(bnds[:-1], bnds[1:]):
                        gsv = GS[:, b0:b1, :]
                        nc.vector.tensor_tensor(gsv, gsv,
                                                BASE[:, b0:b1, :], AO.add)
                        nc.vector.tensor_tensor(gsv, gsv,
                                                GD[:, b0:b1, :], AO.add)
                        nc.scalar.activation(gsv, gsv, AF.Relu)

                    pw = scatter_sw(s, S, GS)
                    finalize_sw(it, s, pw)
                if it < DEPTH:
                    exchange(it)

    nc.compile()
    return nc


_CACHE = {}


def kernel(e, p, gamma, beta, W_i, b_i, W_h, b_h, src, dst, num_nodes):
    e = np.asarray(e, np.float32)
    p = np.asarray(p, np.float32)
    src = np.asarray(src, np.int64)
    dst = np.asarray(dst, np.int64)
    N = int(num_nodes)
    OUT = int(np.asarray(W_i).shape[1])

    plan = Plan(src, dst, N)
    sig = plan.signature()
    if sig not in _CACHE:
        _CACHE[sig] = _build(plan, OUT)
    nc = _CACHE[sig]

    per_core = _host_inputs(plan, e, p, src, dst)
    wts = _weight_inputs(plan, np.asarray(gamma), np.asarray(beta),
                         np.asarray(W_i), np.asarray(b_i),
                         np.asarray(W_h), np.asarray(b_h))
    in_maps = [dict(m, **wts) for m in per_core]

    res = run_bass_kernel_spmd(nc, in_maps, core_ids=list(range(NCORES)))
    fn = np.concatenate([np.asarray(res.results[r]["out_fn"],
                                    np.float32)[:plan.NLOC]
                         for r in range(NCORES)], 0)[:N]
    h = np.concatenate([np.asarray(res.results[r]["out_h"],
                                   np.float32)[:plan.NLOC]
                        for r in range(NCORES)], 0)[:N]
    return np.concatenate([fn, h], axis=1)


# revision 37
# speedup vs baseline: 1.0789x; 1.0692x over previous
"""GCN encoder (edge-wise message passing) on 8 Trainium2 NeuronCores.

Strategy (dst-range sharding):
  - Host: sort edges by dst, shard by dst-range (core r owns nodes
    [r*NLOC, (r+1)*NLOC)), group edges into 128-node windows, pad each
    (window, src-half) group to 128-multiples. Degree / index prep on host.
  - Device: BN stats via ACT-accumulate + tiny AllReduce, folded into W_i.
    Pre-pass computes f_e (feature-major) once, materializes the
    loop-invariant per-edge base = f_e @ Wh_mid + p*w_p to HBM bf16 in a
    partition-contiguous layout, and performs the iter-0 scatter.
    Each iteration: batched per-superwindow dma_gathers pull g_s[src] rows
    from the AllGathered global src-table and g_d[dst] rows from the local
    dst-table; eh = relu(base + g_s + g_d) via two in-place DVE adds + one
    ACT relu; scatter-mean via one-hot matmul into PSUM per 128-node
    window (the one-hot S matrices are built in bulk with stride-0
    broadcast is_equal ops, split across DVE and GPSIMD); then next tables
    g_s|g_d = h @ [Wh_src|Wh_dst] (+b_h), src half AllGathered.
"""
import sys
sys.path.insert(0, "/opt/trn_rl_repo")

import numpy as np
import ml_dtypes
from contextlib import ExitStack

from concourse import bass, bacc, mybir, tile, masks
from concourse.bass_utils import run_bass_kernel_spmd

f32 = mybir.dt.float32
bf16 = mybir.dt.bfloat16
i16 = mybir.dt.int16
i32 = mybir.dt.int32
AO = mybir.AluOpType
AF = mybir.ActivationFunctionType

NCORES = 8
DEPTH = 3
EPS = 1e-5
GW = 4            # windows per superwindow
STAT_SLICE = 1024
S_DVE_BLOCKS = 38   # per-sw S-build blocks on DVE (rest on GPSIMD), /64ths

bfl = ml_dtypes.bfloat16


def _ru(x, m):
    return (x + m - 1) // m * m


class Plan:
    """Host-side preprocessing: sharding, sorting, padding, index layout."""

    def __init__(self, src, dst, N):
        E = src.shape[0]
        self.N, self.E = N, E
        self.NLOC = (N + NCORES - 1) // NCORES
        self.NWIN = (self.NLOC + 127) // 128
        self.NLOCP = self.NWIN * 128
        self.NGLOB = NCORES * self.NLOCP
        # per-core table rows padded even, split into two half-tables that
        # are AllGathered separately (the first half mid-iteration)
        self.NWP = _ru(self.NWIN, 4)
        self.NROW = self.NWP * 128
        self.HR = self.NROW // 2
        assert NCORES * self.HR < 32768   # half-table rows fit int16

        owner = dst // self.NLOC
        local = dst - owner * self.NLOC
        win = local >> 7
        self.ohval_all = (local & 127).astype(np.float32)
        srcloc = src % self.NLOC
        half = (srcloc >= self.HR).astype(np.int64)
        srcrow = (src // self.NLOC) * self.HR + srcloc - half * self.HR
        self.srcrow, self.local, self.owner, self.win, self.half = (
            srcrow, local, owner, win, half)

        key = (owner * self.NWIN + win) * 2 + half
        self.order = np.argsort(key, kind="stable")
        cnt = np.bincount(key, minlength=NCORES * self.NWIN * 2)
        cnt = cnt.reshape(NCORES, self.NWIN, 2)
        self.capA = np.maximum(_ru(cnt[:, :, 0].max(0), 128), 128)
        self.capB = _ru(cnt[:, :, 1].max(0), 128)
        self.cnt = cnt

        # superwindows
        self.NSW = (self.NWIN + GW - 1) // GW
        self.sw_windows = [list(range(s * GW, min((s + 1) * GW, self.NWIN)))
                           for s in range(self.NSW)]
        # slot layout: per sw, [A_w0 B_w0 | A_w1 B_w1 | ...] so each
        # window's chunks are contiguous (single open psum group at a time)
        self.slotA = np.zeros(self.NWIN, np.int64)   # slot offset of A group
        self.slotB = np.zeros(self.NWIN, np.int64)
        self.sw_off = np.zeros(self.NSW + 1, np.int64)
        off = 0
        for s, ws in enumerate(self.sw_windows):
            self.sw_off[s] = off
            a = off
            for w in ws:
                self.slotA[w] = a
                a += self.capA[w]
                self.slotB[w] = a
                a += self.capB[w]
            off = a
        self.sw_off[self.NSW] = off
        self.ES = int(off)
        self.sw_capA = [int(sum(self.capA[w] for w in ws))
                        for ws in self.sw_windows]
        self.sw_capB = [int(sum(self.capB[w] for w in ws))
                        for ws in self.sw_windows]
        self.sw_cap = [a + b for a, b in zip(self.sw_capA, self.sw_capB)]
        self.EMAX4 = _ru(max(int((owner == r).sum()) for r in range(NCORES)), 512)
        self.Q4 = self.EMAX4 // 4

    def signature(self):
        return (self.N, self.E, tuple(self.capA), tuple(self.capB))


def _host_inputs(plan, e, p, src, dst):
    """Build the per-core input arrays."""
    NLOC, NWIN, ES = plan.NLOC, plan.NWIN, plan.ES
    order, cnt = plan.order, plan.cnt
    deg = np.maximum(np.bincount(dst, minlength=plan.N), 1).astype(np.float32)
    invd = 1.0 / deg

    in_maps = []
    pos = 0
    # order slices per (r, w, h) in key order
    slices = {}
    for r in range(NCORES):
        for w in range(NWIN):
            for h in range(2):
                c = int(cnt[r, w, h])
                slices[(r, w, h)] = order[pos:pos + c]
                pos += c
    assert pos == plan.E

    for r in range(NCORES):
        efm = np.zeros((34, ES), np.float32)
        efm[32, :] = 1.0
        gsx = np.zeros(ES, np.int16)
        gdx = np.zeros(ES, np.int16)
        ohv = np.full(ES, -5.0, np.float32)
        for w in range(NWIN):
            for h, base_slot in ((0, plan.slotA[w]), (1, plan.slotB[w])):
                idx = slices[(r, w, h)]
                n = idx.shape[0]
                sl = slice(base_slot, base_slot + n)
                efm[0:32, sl] = e[idx].T
                efm[33, sl] = p[idx, 0]
                gsx[sl] = plan.srcrow[idx]
                gdx[sl] = plan.local[idx]
                ohv[sl] = plan.ohval_all[idx]

        # wrap idxs per superwindow: [16, cap/16] replicated x8
        def wrap(arr):
            out = np.zeros((128, ES // 16), np.int16)
            for s in range(plan.NSW):
                o0, o1 = int(plan.sw_off[s]), int(plan.sw_off[s + 1])
                seg = arr[o0:o1].reshape(-1, 16).T
                out[:, o0 // 16:o1 // 16] = np.tile(seg, (8, 1))
            return out

        soh = ohv.reshape(-1, 128).T.copy()  # [128, ES//128]
        ivl = np.ones(plan.NLOCP, np.float32)
        lo, hi = r * NLOC, min((r + 1) * NLOC, plan.N)
        ivl[:hi - lo] = invd[lo:hi]
        invdeg = ivl.reshape(NWIN, 128).T.copy()  # [128, NWIN]

        mask = plan.owner == np.int64(r)
        er = e[mask]
        epad = np.zeros((plan.EMAX4, 32), np.float32)
        epad[:er.shape[0]] = er
        e4 = epad.reshape(4, plan.Q4, 32).transpose(0, 2, 1).reshape(128, plan.Q4)

        in_maps.append({
            "efm": efm.astype(bfl),
            "gs_idx": wrap(gsx),
            "gd_idx": wrap(gdx),
            "sohb": soh,
            "invdeg": invdeg,
            "e4": e4.astype(bfl),
        })
    return in_maps


def _weight_inputs(plan, gamma, beta, W_i, b_i, W_h, b_h):
    OUT = W_i.shape[1]
    whmid = np.zeros((OUT, 128), np.float32)
    whmid[:, :OUT] = W_h[OUT:2 * OUT]
    wp2 = np.zeros((2, 128), np.float32)
    wp2[1, :OUT] = W_h[2 * OUT]
    whsd = np.zeros((OUT + 1, 256), np.float32)
    whsd[:OUT, 0:OUT] = W_h[0:OUT]
    whsd[:OUT, 128:128 + OUT] = W_h[2 * OUT + 1:3 * OUT + 1]
    whsd[OUT, 128:128 + OUT] = b_h
    return {
        "W_i": W_i.astype(np.float32),
        "b_i": b_i.reshape(OUT, 1).astype(np.float32),
        "gamma": gamma.reshape(32, 1).astype(np.float32),
        "beta": beta.reshape(32, 1).astype(np.float32),
        "whmid": whmid.astype(bfl),
        "wp2": wp2.astype(bfl),
        "whsd": whsd.astype(bfl),
    }


def _build(plan, OUT):
    """Build + compile the SPMD Bass program for this plan."""
    NWIN, NSW, ES = plan.NWIN, plan.NSW, plan.ES
    NLOCP, NGLOB = plan.NLOCP, plan.NGLOB
    IN = 32

    nc = bacc.Bacc("TRN2", target_bir_lowering=False, debug=False,
                   num_devices=NCORES)

    efm = nc.dram_tensor("efm", [34, ES], bf16, kind="ExternalInput")
    gs_idx = nc.dram_tensor("gs_idx", [128, ES // 16], i16, kind="ExternalInput")
    gd_idx = nc.dram_tensor("gd_idx", [128, ES // 16], i16, kind="ExternalInput")
    sohb = nc.dram_tensor("sohb", [128, ES // 128], f32, kind="ExternalInput")
    invdeg = nc.dram_tensor("invdeg", [128, NWIN], f32, kind="ExternalInput")
    e4 = nc.dram_tensor("e4", [128, plan.Q4], bf16, kind="ExternalInput")
    W_i = nc.dram_tensor("W_i", [IN, OUT], f32, kind="ExternalInput")
    b_i = nc.dram_tensor("b_i", [OUT, 1], f32, kind="ExternalInput")
    gamma = nc.dram_tensor("gamma", [IN, 1], f32, kind="ExternalInput")
    beta = nc.dram_tensor("beta", [IN, 1], f32, kind="ExternalInput")
    whmid = nc.dram_tensor("whmid", [OUT, 128], bf16, kind="ExternalInput")
    wp2 = nc.dram_tensor("wp2", [2, 128], bf16, kind="ExternalInput")
    whsd = nc.dram_tensor("whsd", [OUT + 1, 256], bf16, kind="ExternalInput")

    out_fn = nc.dram_tensor("out_fn", [NLOCP, OUT], f32, kind="ExternalOutput")
    out_h = nc.dram_tensor("out_h", [NLOCP, OUT], f32, kind="ExternalOutput")

    inv_E = 1.0 / plan.E

    with tile.TileContext(nc) as tc:
        with ExitStack() as ctx:
            cpool = ctx.enter_context(tc.tile_pool(name="cpool", bufs=1))
            pool = ctx.enter_context(tc.tile_pool(name="pool", bufs=2))
            spool = ctx.enter_context(tc.tile_pool(name="spool", bufs=2))
            psum = ctx.enter_context(tc.tile_pool(name="psum", bufs=2,
                                                  space="PSUM"))
            dram = ctx.enter_context(tc.tile_pool(name="dram", bufs=1,
                                                  space="DRAM"))

            # ---- constants ----
            iota_i = cpool.tile([128, 128], i32)
            nc.gpsimd.iota(iota_i[:], pattern=[[1, 128]], base=0,
                           channel_multiplier=0)
            iota_b = cpool.tile([128, 128], bf16)
            nc.vector.tensor_copy(iota_b[:], iota_i[:])
            ident = cpool.tile([128, 128], f32)
            masks.make_identity(nc, ident[:])

            whmid_t = cpool.tile([OUT, 128], bf16)
            nc.sync.dma_start(whmid_t[:], whmid[:])
            wp2_t = cpool.tile([34, 128], bf16)
            nc.sync.dma_start(wp2_t[32:34, :], wp2[:])
            whsd_t = cpool.tile([OUT + 1, 256], bf16)
            nc.sync.dma_start(whsd_t[:], whsd[:])
            invdeg_t = cpool.tile([128, NWIN], f32)
            nc.sync.dma_start(invdeg_t[:], invdeg[:])

            # ---- BN stats: per-core partial sums of e, e^2 ----
            nsl = (plan.Q4 + STAT_SLICE - 1) // STAT_SLICE
            parts = cpool.tile([128, 2 * nsl], f32)
            for s in range(nsl):
                c0, c1 = s * STAT_SLICE, min((s + 1) * STAT_SLICE, plan.Q4)
                esl = spool.tile([128, STAT_SLICE], bf16, tag="esl")
                nc.sync.dma_start(esl[:, :c1 - c0], e4[:, c0:c1])
                junk = spool.tile([128, STAT_SLICE], bf16, tag="junk")
                nc.scalar.activation(junk[:, :c1 - c0], esl[:, :c1 - c0],
                                     AF.Copy, accum_out=parts[:, s:s + 1])
                nc.scalar.activation(junk[:, :c1 - c0], esl[:, :c1 - c0],
                                     AF.Square,
                                     accum_out=parts[:, nsl + s:nsl + s + 1])
            sums = cpool.tile([128, 2], f32)
            junk2 = cpool.tile([128, nsl], f32)
            nc.scalar.activation(junk2[:], parts[:, 0:nsl], AF.Copy,
                                 accum_out=sums[:, 0:1])
            nc.scalar.activation(junk2[:], parts[:, nsl:2 * nsl], AF.Copy,
                                 accum_out=sums[:, 1:2])
            ar_in = dram.tile([128, 2], f32)
            ar_out = dram.tile([128, 2], f32)
            nc.sync.dma_start(ar_in[:], sums[:])
            nc.gpsimd.collective_compute(
                "AllReduce", AO.add, replica_groups=[list(range(NCORES))],
                ins=[ar_in.opt()], outs=[ar_out.opt()])
            g4 = cpool.tile([32, 4, 2], f32)
            nc.sync.dma_start(
                g4[:], ar_out[:].rearrange("(g p) k -> p g k", g=4))
            t1 = cpool.tile([32, 2], f32)
            t2 = cpool.tile([32, 2], f32)
            tot = cpool.tile([32, 2], f32)
            nc.vector.tensor_tensor(t1[:], g4[:, 0, :], g4[:, 1, :], AO.add)
            nc.vector.tensor_tensor(t2[:], g4[:, 2, :], g4[:, 3, :], AO.add)
            nc.vector.tensor_tensor(tot[:], t1[:], t2[:], AO.add)
            mu = cpool.tile([32, 1], f32)
            nc.vector.tensor_scalar(mu[:], tot[:, 0:1], inv_E, None, op0=AO.mult)
            ms = cpool.tile([32, 1], f32)
            nc.vector.tensor_scalar(ms[:], tot[:, 1:2], inv_E, None, op0=AO.mult)
            var = cpool.tile([32, 1], f32)
            mu2 = cpool.tile([32, 1], f32)
            nc.vector.tensor_tensor(mu2[:], mu[:], mu[:], AO.mult)
            nc.vector.tensor_tensor(var[:], ms[:], mu2[:], AO.subtract)
            epsb = cpool.tile([32, 1], f32)
            nc.vector.memset(epsb[:], EPS)
            std = cpool.tile([32, 1], f32)
            nc.scalar.activation(std[:], var[:], AF.Sqrt, bias=epsb[:])
            rstd = cpool.tile([32, 1], f32)
            nc.vector.reciprocal(rstd[:], std[:])
            gam_t = cpool.tile([32, 1], f32)
            nc.sync.dma_start(gam_t[:], gamma[:])
            bet_t = cpool.tile([32, 1], f32)
            nc.sync.dma_start(bet_t[:], beta[:])
            a_t = cpool.tile([32, 1], f32)
            nc.vector.tensor_tensor(a_t[:], gam_t[:], rstd[:], AO.mult)
            nma = cpool.tile([32, 1], f32)
            nc.vector.scalar_tensor_tensor(nma[:], mu[:], -1.0, a_t[:],
                                           op0=AO.mult, op1=AO.mult)
            c_t = cpool.tile([32, 1], f32)
            nc.vector.tensor_tensor(c_t[:], bet_t[:], nma[:], AO.add)

            wi_t = cpool.tile([32, OUT], f32)
            nc.sync.dma_start(wi_t[:], W_i[:])
            wif = cpool.tile([32, OUT], f32)
            nc.vector.tensor_scalar(wif[:], wi_t[:], a_t[:], None, op0=AO.mult)
            bi_t = cpool.tile([OUT, 1], f32)
            nc.sync.dma_start(bi_t[:], b_i[:])
            pb0 = psum.tile([128, 512], f32, tag="pfin")
            nc.tensor.matmul(pb0[:OUT, 0:1], wif[:], c_t[:], start=True,
                             stop=True)
            bcol = cpool.tile([OUT, 1], f32)
            nc.vector.tensor_tensor(bcol[:], pb0[:OUT, 0:1], bi_t[:], AO.add)
            scr = dram.tile([OUT, 1], f32)
            nc.sync.dma_start(scr[:], bcol[:])
            # wiaug padded to 128 cols so fee matmuls define full psum banks
            wiaug = cpool.tile([33, 128], bf16)
            nc.vector.memset(wiaug[:], 0.0)
            nc.vector.tensor_copy(wiaug[0:32, :OUT], wif[:])
            nc.gpsimd.dma_start(wiaug[32:33, :OUT],
                                scr[:].rearrange("a b -> b a"))

            # ---- DRAM intermediates ----
            # base, partition-contiguous: col b*128+f on partition p holds
            # base[slot b*128+p, f]
            baseH2 = dram.tile([128, ES], bf16, name="baseH2")
            HR = plan.HR
            tlgs = [dram.tile([plan.NROW, 128], bf16, name=f"tlgs{k}",
                              tag=f"tlgs{k}") for k in range(DEPTH)]
            tlgd = [dram.tile([NLOCP, 128], bf16, name=f"tlgd{k}",
                              tag=f"tlgd{k}") for k in range(DEPTH)]
            tgA = [dram.tile([NCORES * HR, 128], bf16, name=f"tgA{k}",
                             tag=f"tgA{k}") for k in range(DEPTH)]
            tgB = [dram.tile([NCORES * HR, 128], bf16, name=f"tgB{k}",
                             tag=f"tgB{k}") for k in range(DEPTH)]
            # last superwindow whose windows complete table rows [0, HR)
            AG1SW = ((HR // 128) + GW - 1) // GW - 1

            def ag_half(e, h):
                src_ap = (tlgs[e][0:HR, :] if h == 0
                          else tlgs[e][HR:plan.NROW, :])
                out_t = tgA[e] if h == 0 else tgB[e]
                nc.gpsimd.collective_compute(
                    "AllGather", AO.bypass,
                    replica_groups=[list(range(NCORES))],
                    ins=[src_ap.opt()], outs=[out_t.opt()])

            def build_S(s):
                """One-hot scatter matrices for superwindow s (bulk build)."""
                cap = plan.sw_cap[s]
                nblk = cap // 128
                o0 = int(plan.sw_off[s])
                maxnblk = max(plan.sw_cap) // 128
                sohc = pool.tile([128, maxnblk], f32, tag="sohc")
                nc.sync.dma_start(sohc[:, :nblk],
                                  sohb[:, o0 // 128:o0 // 128 + nblk])
                S = pool.tile([128, nblk, 128], bf16, tag="S")
                for b in range(nblk):
                    nc.vector.tensor_scalar(S[:, b, :], iota_b[:],
                                            sohc[:, b:b + 1], None,
                                            op0=AO.is_equal)
                return S

            def window_chunks(s, w):
                """(block) list of window w within superwindow s."""
                o = int(plan.sw_off[s])
                b0 = (int(plan.slotA[w]) - o) // 128
                nb = (int(plan.capA[w]) + int(plan.capB[w])) // 128
                return [b0 + c for c in range(nb)]

            def scatter_sw(s, S, src_tile):
                """One-hot scatter of src_tile rows into per-window psum."""
                pw = psum.tile([128, 512], f32, tag="pw")
                for wl, w in enumerate(plan.sw_windows[s]):
                    chunks = window_chunks(s, w)
                    for ci, b in enumerate(chunks):
                        nc.tensor.matmul(
                            pw[:, wl * 128:wl * 128 + OUT], S[:, b, :],
                            src_tile[:, b, :OUT], start=(ci == 0),
                            stop=(ci == len(chunks) - 1))
                return pw

            def finalize_sw(it, s, pw):
                """pw[:, wl*128:...] holds the scatter sums per window."""
                ws = plan.sw_windows[s]
                nw = len(ws)
                w0 = ws[0]
                hC = pool.tile([128, GW, 128], f32, tag="hC", name="hC")
                ttabs = (pool.tile([128, GW, 256], bf16, tag="ttabs",
                                   name="ttabs")
                         if it < DEPTH else None)
                for wl, w in enumerate(ws):
                    pwv = pw[:, wl * 128:wl * 128 + OUT]
                    nc.vector.tensor_scalar(hC[:, wl, :OUT], pwv,
                                            invdeg_t[:, w:w + 1], None,
                                            op0=AO.mult)
                    if it < DEPTH:
                        pf = psum.tile([128, 512], f32, tag="pfin")
                        nc.tensor.transpose(pf[:OUT, :128], hC[:, wl, :OUT],
                                            ident[:])
                        hT = pool.tile([OUT + 1, 128], bf16, tag="hT")
                        nc.vector.memset(hT[:], 1.0)
                        nc.vector.tensor_copy(hT[:OUT, :], pf[:OUT, :128])
                        nc.tensor.matmul(pf[:, 128:384], hT[:], whsd_t[:],
                                         start=True, stop=True)
                        nc.scalar.copy(ttabs[:, wl, :], pf[:, 128:384])
                rows = slice(w0 * 128, (w0 + nw) * 128)
                if it == 0:
                    nc.sync.dma_start(
                        out_fn[rows, :].rearrange("(b p) f -> p b f", p=128),
                        hC[:, :nw, :OUT])
                if it == DEPTH:
                    nc.sync.dma_start(
                        out_h[rows, :].rearrange("(b p) f -> p b f", p=128),
                        hC[:, :nw, :OUT])
                    return
                nc.sync.dma_start(
                    tlgs[it][rows, :].rearrange("(b p) f -> p b f", p=128),
                    ttabs[:, :nw, 0:128])
                nc.sync.dma_start(
                    tlgd[it][rows, :].rearrange("(b p) f -> p b f", p=128),
                    ttabs[:, :nw, 128:256])

            # ---- pre-pass + iter 0 ----
            for s in range(NSW):
                cap = plan.sw_cap[s]
                nblk = cap // 128
                o0 = int(plan.sw_off[s])
                efm_t = pool.tile([34, cap], bf16, tag="big0")
                nc.sync.dma_start(efm_t[:], efm[:, o0:o0 + cap])
                S = build_S(s)
                feT = pool.tile([OUT, cap], bf16, tag="big1")

                for gi, g0 in enumerate(range(0, cap, 512)):
                    g1 = min(g0 + 512, cap)
                    p1 = psum.tile([128, 512], f32, tag="pA")
                    nc.tensor.matmul(p1[:OUT, :g1 - g0], wiaug[:, :OUT],
                                     efm_t[0:33, g0:g1], start=True, stop=True)
                    if gi % 2 == 0:
                        nc.scalar.activation(feT[:, g0:g1], p1[:OUT, :g1 - g0],
                                             AF.Relu)
                    else:
                        nc.vector.tensor_scalar(feT[:, g0:g1],
                                                p1[:OUT, :g1 - g0], 0.0, None,
                                                op0=AO.max)

                # per-window chunk bookkeeping for interleaved scatter
                block2w = {}
                wlen, wdone = {}, {}
                for wl, w in enumerate(plan.sw_windows[s]):
                    chunks = window_chunks(s, w)
                    wlen[w] = len(chunks)
                    wdone[w] = 0
                    for b in chunks:
                        block2w[b] = (wl, w)
                pw = psum.tile([128, 512], f32, tag="pw")

                for gi, g0 in enumerate(range(0, cap, 512)):
                    g1 = min(g0 + 512, cap)
                    nb = (g1 - g0) // 128
                    pb = psum.tile([128, 512], f32, tag="pbase")
                    pf = psum.tile([128, 512], f32, tag="pA")
                    for ci in range(nb):
                        sl = slice(g0 + ci * 128, g0 + (ci + 1) * 128)
                        cl = slice(ci * 128, (ci + 1) * 128)
                        nc.tensor.matmul(pb[:, cl], feT[:, sl], whmid_t[:],
                                         start=True, stop=False)
                        nc.tensor.matmul(pb[:, cl], efm_t[32:34, sl],
                                         wp2_t[32:34, :], start=False,
                                         stop=True)
                        nc.tensor.matmul(pf[:, cl], efm_t[0:33, sl], wiaug[:],
                                         start=True, stop=True)
                    basec = pool.tile([128, 512], bf16, tag="basec", bufs=3)
                    fee = pool.tile([128, 4, 128], bf16, tag="fee", bufs=3)
                    fv = fee[:].rearrange("p b f -> p (b f)")
                    if gi % 2 == 0:
                        nc.vector.tensor_copy(basec[:, :g1 - g0],
                                              pb[:, :g1 - g0])
                        nc.scalar.activation(fv[:, :g1 - g0], pf[:, :g1 - g0],
                                             AF.Relu)
                    else:
                        nc.scalar.copy(basec[:, :g1 - g0], pb[:, :g1 - g0])
                        nc.vector.tensor_scalar(fv[:, :g1 - g0],
                                                pf[:, :g1 - g0],
                                                0.0, None, op0=AO.max)
                    nc.sync.dma_start(baseH2[:, o0 + g0:o0 + g1],
                                      basec[:, :g1 - g0])
                    for ci in range(nb):
                        b = g0 // 128 + ci
                        wl, w = block2w[b]
                        k = wdone[w]
                        wdone[w] = k + 1
                        nc.tensor.matmul(
                            pw[:, wl * 128:wl * 128 + OUT], S[:, b, :],
                            fee[:, ci, :OUT], start=(k == 0),
                            stop=(k == wlen[w] - 1))
                finalize_sw(0, s, pw)
                if s == AG1SW:
                    ag_half(0, 0)
            ag_half(0, 1)

            # ---- iterations 1..DEPTH ----
            for it in range(1, DEPTH + 1):
                for s in range(NSW):
                    cap = plan.sw_cap[s]
                    nblk = cap // 128
                    capA = plan.sw_capA[s]
                    capB = plan.sw_capB[s]
                    o0 = int(plan.sw_off[s])
                    GS = pool.tile([128, nblk, 128], bf16, tag="big0")
                    GD = pool.tile([128, nblk, 128], bf16, tag="big1")
                    BASE = pool.tile([128, nblk, 128], bf16, tag="big2")
                    nc.sync.dma_start(
                        BASE[:].rearrange("p b f -> p (b f)"),
                        baseH2[:, o0:o0 + cap])
                    S = build_S(s)
                    gsix = pool.tile([128, cap // 16], i16, tag="gsix")
                    nc.sync.dma_start(gsix[:],
                                      gs_idx[:, o0 // 16:(o0 + cap) // 16])
                    gdix = pool.tile([128, cap // 16], i16, tag="gdix")
                    nc.sync.dma_start(gdix[:],
                                      gd_idx[:, o0 // 16:(o0 + cap) // 16])

                    GPC = 1024   # gather piece size (fits the SWDGE ring)
                    for q0 in range(0, cap, GPC):
                        m = min(GPC, cap - q0)
                        nc.gpsimd.dma_gather(
                            GD[:, q0 // 128:(q0 + m) // 128, :],
                            tlgd[it - 1][:], gdix[:, q0 // 16:(q0 + m) // 16],
                            m, m, 128, elem_step=128)
                    for w in plan.sw_windows[s]:
                        regs = ((int(plan.slotA[w]) - o0, int(plan.capA[w]),
                                 tgA[it - 1][:]),
                                (int(plan.slotB[w]) - o0, int(plan.capB[w]),
                                 tgB[it - 1][:]))
                        for r0, ln, srcv in regs:
                            for q0 in range(r0, r0 + ln, GPC):
                                m = min(GPC, r0 + ln - q0)
                                nc.gpsimd.dma_gather(
                                    GS[:, q0 // 128:(q0 + m) // 128, :], srcv,
                                    gsix[:, q0 // 16:(q0 + m) // 16],
                                    m, m, 128, elem_step=128)

                    qs = max(1, nblk // 4)
                    bnds = list(range(0, nblk, qs)) + [nblk]
                    for b0, b1 in zip(bnds[:-1], bnds[1:]):
                        gsv = GS[:, b0:b1, :]
                        nc.vector.tensor_tensor(gsv, gsv,
                                                BASE[:, b0:b1, :], AO.add)
                        nc.vector.tensor_tensor(gsv, gsv,
                                                GD[:, b0:b1, :], AO.add)
                        nc.scalar.activation(gsv, gsv, AF.Relu)

                    pw = scatter_sw(s, S, GS)
                    finalize_sw(it, s, pw)
                    if it < DEPTH and s == AG1SW:
                        ag_half(it, 0)
                if it < DEPTH:
                    ag_half(it, 1)

    nc.compile()
    return nc


_CACHE = {}


def kernel(e, p, gamma, beta, W_i, b_i, W_h, b_h, src, dst, num_nodes):
    e = np.asarray(e, np.float32)
    p = np.asarray(p, np.float32)
    src = np.asarray(src, np.int64)
    dst = np.asarray(dst, np.int64)
    N = int(num_nodes)
    OUT = int(np.asarray(W_i).shape[1])

    plan = Plan(src, dst, N)
    sig = plan.signature()
    if sig not in _CACHE:
        _CACHE[sig] = _build(plan, OUT)
    nc = _CACHE[sig]

    per_core = _host_inputs(plan, e, p, src, dst)
    wts = _weight_inputs(plan, np.asarray(gamma), np.asarray(beta),
                         np.asarray(W_i), np.asarray(b_i),
                         np.asarray(W_h), np.asarray(b_h))
    in_maps = [dict(m, **wts) for m in per_core]

    res = run_bass_kernel_spmd(nc, in_maps, core_ids=list(range(NCORES)))
    fn = np.concatenate([np.asarray(res.results[r]["out_fn"],
                                    np.float32)[:plan.NLOC]
                         for r in range(NCORES)], 0)[:N]
    h = np.concatenate([np.asarray(res.results[r]["out_h"],
                                   np.float32)[:plan.NLOC]
                        for r in range(NCORES)], 0)[:N]
    return np.concatenate([fn, h], axis=1)


# revision 40
# speedup vs baseline: 1.1241x; 1.0419x over previous
"""GCN encoder (edge-wise message passing) on 8 Trainium2 NeuronCores.

Strategy (dst-range sharding):
  - Host: sort edges by dst, shard by dst-range (core r owns nodes
    [r*NLOC, (r+1)*NLOC)), group edges into 128-node windows, pad each
    (window, src-half) group to 128-multiples. Degree / index prep on host.
  - Device: BN stats via ACT-accumulate + tiny AllReduce, folded into W_i.
    Pre-pass computes f_e (feature-major) once, materializes the
    loop-invariant per-edge base = f_e @ Wh_mid + p*w_p to HBM bf16 in a
    partition-contiguous layout, and performs the iter-0 scatter.
    Each iteration: batched per-superwindow dma_gathers pull g_s[src] rows
    from the AllGathered global src-table and g_d[dst] rows from the local
    dst-table; eh = relu(base + g_s + g_d) via two in-place DVE adds + one
    ACT relu; scatter-mean via one-hot matmul into PSUM per 128-node
    window (the one-hot S matrices are built in bulk with stride-0
    broadcast is_equal ops, split across DVE and GPSIMD); then next tables
    g_s|g_d = h @ [Wh_src|Wh_dst] (+b_h), src half AllGathered.
"""
import sys
sys.path.insert(0, "/opt/trn_rl_repo")

import numpy as np
import ml_dtypes
from contextlib import ExitStack

from concourse import bass, bacc, mybir, tile, masks
from concourse.bass_utils import run_bass_kernel_spmd

f32 = mybir.dt.float32
bf16 = mybir.dt.bfloat16
i16 = mybir.dt.int16
i32 = mybir.dt.int32
AO = mybir.AluOpType
AF = mybir.ActivationFunctionType

NCORES = 8
DEPTH = 3
EPS = 1e-5
GW = 4            # windows per superwindow
STAT_SLICE = 1024
S_DVE_BLOCKS = 38   # per-sw S-build blocks on DVE (rest on GPSIMD), /64ths

bfl = ml_dtypes.bfloat16


def _ru(x, m):
    return (x + m - 1) // m * m


class Plan:
    """Host-side preprocessing: sharding, sorting, padding, index layout."""

    def __init__(self, src, dst, N):
        E = src.shape[0]
        self.N, self.E = N, E
        self.NLOC = (N + NCORES - 1) // NCORES
        self.NWIN = (self.NLOC + 127) // 128
        self.NLOCP = self.NWIN * 128
        self.NGLOB = NCORES * self.NLOCP
        # per-core table rows padded even, split into two half-tables that
        # are AllGathered separately (the first half mid-iteration)
        self.NWP = _ru(self.NWIN, 4)
        self.NROW = self.NWP * 128
        self.QR = self.NROW // 4
        assert NCORES * self.QR < 32768   # quarter-table rows fit int16

        owner = dst // self.NLOC
        local = dst - owner * self.NLOC
        win = local >> 7
        self.ohval_all = (local & 127).astype(np.float32)
        srcloc = src % self.NLOC
        quarter = np.minimum(srcloc // self.QR, 3).astype(np.int64)
        srcrow = (src // self.NLOC) * self.QR + srcloc - quarter * self.QR
        self.srcrow, self.local, self.owner, self.win, self.quarter = (
            srcrow, local, owner, win, quarter)

        key = (owner * self.NWIN + win) * 4 + quarter
        self.order = np.argsort(key, kind="stable")
        cnt = np.bincount(key, minlength=NCORES * self.NWIN * 4)
        cnt = cnt.reshape(NCORES, self.NWIN, 4)
        self.capQ = _ru(cnt.max(0), 128)          # [NWIN, 4]
        self.capQ[:, 0] = np.maximum(self.capQ[:, 0], 128)
        self.cnt = cnt

        # superwindows
        self.NSW = (self.NWIN + GW - 1) // GW
        self.sw_windows = [list(range(s * GW, min((s + 1) * GW, self.NWIN)))
                           for s in range(self.NSW)]
        # slot layout: per sw, [Q0_w0..Q3_w0 | Q0_w1..Q3_w1 | ...] so each
        # window's chunks are contiguous (single open psum group at a time)
        self.slotQ = np.zeros((self.NWIN, 4), np.int64)
        self.sw_off = np.zeros(self.NSW + 1, np.int64)
        off = 0
        for s, ws in enumerate(self.sw_windows):
            self.sw_off[s] = off
            a = off
            for w in ws:
                for q in range(4):
                    self.slotQ[w, q] = a
                    a += self.capQ[w, q]
            off = a
        self.sw_off[self.NSW] = off
        self.ES = int(off)
        self.sw_cap = [int(sum(self.capQ[w].sum() for w in ws))
                       for ws in self.sw_windows]
        self.EMAX4 = _ru(max(int((owner == r).sum()) for r in range(NCORES)), 512)
        self.Q4 = self.EMAX4 // 4

    def signature(self):
        return (self.N, self.E, tuple(self.capQ.ravel()))


def _host_inputs(plan, e, p, src, dst):
    """Build the per-core input arrays."""
    NLOC, NWIN, ES = plan.NLOC, plan.NWIN, plan.ES
    order, cnt = plan.order, plan.cnt
    deg = np.maximum(np.bincount(dst, minlength=plan.N), 1).astype(np.float32)
    invd = 1.0 / deg

    in_maps = []
    pos = 0
    # order slices per (r, w, h) in key order
    slices = {}
    for r in range(NCORES):
        for w in range(NWIN):
            for h in range(4):
                c = int(cnt[r, w, h])
                slices[(r, w, h)] = order[pos:pos + c]
                pos += c
    assert pos == plan.E

    for r in range(NCORES):
        efm = np.zeros((34, ES), np.float32)
        efm[32, :] = 1.0
        gsx = np.zeros(ES, np.int16)
        gdx = np.zeros(ES, np.int16)
        ohv = np.full(ES, -5.0, np.float32)
        for w in range(NWIN):
            for h, base_slot in enumerate(plan.slotQ[w]):
                idx = slices[(r, w, h)]
                n = idx.shape[0]
                sl = slice(base_slot, base_slot + n)
                efm[0:32, sl] = e[idx].T
                efm[33, sl] = p[idx, 0]
                gsx[sl] = plan.srcrow[idx]
                gdx[sl] = plan.local[idx]
                ohv[sl] = plan.ohval_all[idx]

        # wrap idxs per superwindow: [16, cap/16] replicated x8
        def wrap(arr):
            out = np.zeros((128, ES // 16), np.int16)
            for s in range(plan.NSW):
                o0, o1 = int(plan.sw_off[s]), int(plan.sw_off[s + 1])
                seg = arr[o0:o1].reshape(-1, 16).T
                out[:, o0 // 16:o1 // 16] = np.tile(seg, (8, 1))
            return out

        soh = ohv.reshape(-1, 128).T.copy()  # [128, ES//128]
        ivl = np.ones(plan.NLOCP, np.float32)
        lo, hi = r * NLOC, min((r + 1) * NLOC, plan.N)
        ivl[:hi - lo] = invd[lo:hi]
        invdeg = ivl.reshape(NWIN, 128).T.copy()  # [128, NWIN]

        mask = plan.owner == np.int64(r)
        er = e[mask]
        epad = np.zeros((plan.EMAX4, 32), np.float32)
        epad[:er.shape[0]] = er
        e4 = epad.reshape(4, plan.Q4, 32).transpose(0, 2, 1).reshape(128, plan.Q4)

        in_maps.append({
            "efm": efm.astype(bfl),
            "gs_idx": wrap(gsx),
            "gd_idx": wrap(gdx),
            "sohb": soh,
            "invdeg": invdeg,
            "e4": e4.astype(bfl),
        })
    return in_maps


def _weight_inputs(plan, gamma, beta, W_i, b_i, W_h, b_h):
    OUT = W_i.shape[1]
    whmid = np.zeros((OUT, 128), np.float32)
    whmid[:, :OUT] = W_h[OUT:2 * OUT]
    wp2 = np.zeros((2, 128), np.float32)
    wp2[1, :OUT] = W_h[2 * OUT]
    whsd = np.zeros((OUT + 1, 256), np.float32)
    whsd[:OUT, 0:OUT] = W_h[0:OUT]
    whsd[:OUT, 128:128 + OUT] = W_h[2 * OUT + 1:3 * OUT + 1]
    whsd[OUT, 128:128 + OUT] = b_h
    return {
        "W_i": W_i.astype(np.float32),
        "b_i": b_i.reshape(OUT, 1).astype(np.float32),
        "gamma": gamma.reshape(32, 1).astype(np.float32),
        "beta": beta.reshape(32, 1).astype(np.float32),
        "whmid": whmid.astype(bfl),
        "wp2": wp2.astype(bfl),
        "whsd": whsd.astype(bfl),
    }


def _build(plan, OUT):
    """Build + compile the SPMD Bass program for this plan."""
    NWIN, NSW, ES = plan.NWIN, plan.NSW, plan.ES
    NLOCP, NGLOB = plan.NLOCP, plan.NGLOB
    IN = 32

    nc = bacc.Bacc("TRN2", target_bir_lowering=False, debug=False,
                   num_devices=NCORES)

    efm = nc.dram_tensor("efm", [34, ES], bf16, kind="ExternalInput")
    gs_idx = nc.dram_tensor("gs_idx", [128, ES // 16], i16, kind="ExternalInput")
    gd_idx = nc.dram_tensor("gd_idx", [128, ES // 16], i16, kind="ExternalInput")
    sohb = nc.dram_tensor("sohb", [128, ES // 128], f32, kind="ExternalInput")
    invdeg = nc.dram_tensor("invdeg", [128, NWIN], f32, kind="ExternalInput")
    e4 = nc.dram_tensor("e4", [128, plan.Q4], bf16, kind="ExternalInput")
    W_i = nc.dram_tensor("W_i", [IN, OUT], f32, kind="ExternalInput")
    b_i = nc.dram_tensor("b_i", [OUT, 1], f32, kind="ExternalInput")
    gamma = nc.dram_tensor("gamma", [IN, 1], f32, kind="ExternalInput")
    beta = nc.dram_tensor("beta", [IN, 1], f32, kind="ExternalInput")
    whmid = nc.dram_tensor("whmid", [OUT, 128], bf16, kind="ExternalInput")
    wp2 = nc.dram_tensor("wp2", [2, 128], bf16, kind="ExternalInput")
    whsd = nc.dram_tensor("whsd", [OUT + 1, 256], bf16, kind="ExternalInput")

    out_fn = nc.dram_tensor("out_fn", [NLOCP, OUT], f32, kind="ExternalOutput")
    out_h = nc.dram_tensor("out_h", [NLOCP, OUT], f32, kind="ExternalOutput")

    inv_E = 1.0 / plan.E

    with tile.TileContext(nc) as tc:
        with ExitStack() as ctx:
            cpool = ctx.enter_context(tc.tile_pool(name="cpool", bufs=1))
            pool = ctx.enter_context(tc.tile_pool(name="pool", bufs=2))
            spool = ctx.enter_context(tc.tile_pool(name="spool", bufs=2))
            psum = ctx.enter_context(tc.tile_pool(name="psum", bufs=2,
                                                  space="PSUM"))
            dram = ctx.enter_context(tc.tile_pool(name="dram", bufs=1,
                                                  space="DRAM"))

            # ---- constants ----
            iota_i = cpool.tile([128, 128], i32)
            nc.gpsimd.iota(iota_i[:], pattern=[[1, 128]], base=0,
                           channel_multiplier=0)
            iota_b = cpool.tile([128, 128], bf16)
            nc.vector.tensor_copy(iota_b[:], iota_i[:])
            ident = cpool.tile([128, 128], f32)
            masks.make_identity(nc, ident[:])

            whmid_t = cpool.tile([OUT, 128], bf16)
            nc.sync.dma_start(whmid_t[:], whmid[:])
            wp2_t = cpool.tile([34, 128], bf16)
            nc.sync.dma_start(wp2_t[32:34, :], wp2[:])
            whsd_t = cpool.tile([OUT + 1, 256], bf16)
            nc.sync.dma_start(whsd_t[:], whsd[:])
            invdeg_t = cpool.tile([128, NWIN], f32)
            nc.sync.dma_start(invdeg_t[:], invdeg[:])

            # ---- BN stats: per-core partial sums of e, e^2 ----
            nsl = (plan.Q4 + STAT_SLICE - 1) // STAT_SLICE
            parts = cpool.tile([128, 2 * nsl], f32)
            for s in range(nsl):
                c0, c1 = s * STAT_SLICE, min((s + 1) * STAT_SLICE, plan.Q4)
                esl = spool.tile([128, STAT_SLICE], bf16, tag="esl")
                nc.sync.dma_start(esl[:, :c1 - c0], e4[:, c0:c1])
                junk = spool.tile([128, STAT_SLICE], bf16, tag="junk")
                nc.scalar.activation(junk[:, :c1 - c0], esl[:, :c1 - c0],
                                     AF.Copy, accum_out=parts[:, s:s + 1])
                nc.scalar.activation(junk[:, :c1 - c0], esl[:, :c1 - c0],
                                     AF.Square,
                                     accum_out=parts[:, nsl + s:nsl + s + 1])
            sums = cpool.tile([128, 2], f32)
            junk2 = cpool.tile([128, nsl], f32)
            nc.scalar.activation(junk2[:], parts[:, 0:nsl], AF.Copy,
                                 accum_out=sums[:, 0:1])
            nc.scalar.activation(junk2[:], parts[:, nsl:2 * nsl], AF.Copy,
                                 accum_out=sums[:, 1:2])
            ar_in = dram.tile([128, 2], f32)
            ar_out = dram.tile([128, 2], f32)
            nc.sync.dma_start(ar_in[:], sums[:])
            nc.gpsimd.collective_compute(
                "AllReduce", AO.add, replica_groups=[list(range(NCORES))],
                ins=[ar_in.opt()], outs=[ar_out.opt()])
            g4 = cpool.tile([32, 4, 2], f32)
            nc.sync.dma_start(
                g4[:], ar_out[:].rearrange("(g p) k -> p g k", g=4))
            t1 = cpool.tile([32, 2], f32)
            t2 = cpool.tile([32, 2], f32)
            tot = cpool.tile([32, 2], f32)
            nc.vector.tensor_tensor(t1[:], g4[:, 0, :], g4[:, 1, :], AO.add)
            nc.vector.tensor_tensor(t2[:], g4[:, 2, :], g4[:, 3, :], AO.add)
            nc.vector.tensor_tensor(tot[:], t1[:], t2[:], AO.add)
            mu = cpool.tile([32, 1], f32)
            nc.vector.tensor_scalar(mu[:], tot[:, 0:1], inv_E, None, op0=AO.mult)
            ms = cpool.tile([32, 1], f32)
            nc.vector.tensor_scalar(ms[:], tot[:, 1:2], inv_E, None, op0=AO.mult)
            var = cpool.tile([32, 1], f32)
            mu2 = cpool.tile([32, 1], f32)
            nc.vector.tensor_tensor(mu2[:], mu[:], mu[:], AO.mult)
            nc.vector.tensor_tensor(var[:], ms[:], mu2[:], AO.subtract)
            epsb = cpool.tile([32, 1], f32)
            nc.vector.memset(epsb[:], EPS)
            std = cpool.tile([32, 1], f32)
            nc.scalar.activation(std[:], var[:], AF.Sqrt, bias=epsb[:])
            rstd = cpool.tile([32, 1], f32)
            nc.vector.reciprocal(rstd[:], std[:])
            gam_t = cpool.tile([32, 1], f32)
            nc.sync.dma_start(gam_t[:], gamma[:])
            bet_t = cpool.tile([32, 1], f32)
            nc.sync.dma_start(bet_t[:], beta[:])
            a_t = cpool.tile([32, 1], f32)
            nc.vector.tensor_tensor(a_t[:], gam_t[:], rstd[:], AO.mult)
            nma = cpool.tile([32, 1], f32)
            nc.vector.scalar_tensor_tensor(nma[:], mu[:], -1.0, a_t[:],
                                           op0=AO.mult, op1=AO.mult)
            c_t = cpool.tile([32, 1], f32)
            nc.vector.tensor_tensor(c_t[:], bet_t[:], nma[:], AO.add)

            wi_t = cpool.tile([32, OUT], f32)
            nc.sync.dma_start(wi_t[:], W_i[:])
            wif = cpool.tile([32, OUT], f32)
            nc.vector.tensor_scalar(wif[:], wi_t[:], a_t[:], None, op0=AO.mult)
            bi_t = cpool.tile([OUT, 1], f32)
            nc.sync.dma_start(bi_t[:], b_i[:])
            pb0 = psum.tile([128, 512], f32, tag="pfin")
            nc.tensor.matmul(pb0[:OUT, 0:1], wif[:], c_t[:], start=True,
                             stop=True)
            bcol = cpool.tile([OUT, 1], f32)
            nc.vector.tensor_tensor(bcol[:], pb0[:OUT, 0:1], bi_t[:], AO.add)
            scr = dram.tile([OUT, 1], f32)
            nc.sync.dma_start(scr[:], bcol[:])
            # wiaug padded to 128 cols so fee matmuls define full psum banks
            wiaug = cpool.tile([33, 128], bf16)
            nc.vector.memset(wiaug[:], 0.0)
            nc.vector.tensor_copy(wiaug[0:32, :OUT], wif[:])
            nc.gpsimd.dma_start(wiaug[32:33, :OUT],
                                scr[:].rearrange("a b -> b a"))

            # ---- DRAM intermediates ----
            # base, partition-contiguous: col b*128+f on partition p holds
            # base[slot b*128+p, f]
            baseH2 = dram.tile([128, ES], bf16, name="baseH2")
            QR = plan.QR
            tlgs = [dram.tile([plan.NROW, 128], bf16, name=f"tlgs{k}",
                              tag=f"tlgs{k}") for k in range(DEPTH)]
            tlgd = [dram.tile([NLOCP, 128], bf16, name=f"tlgd{k}",
                              tag=f"tlgd{k}") for k in range(DEPTH)]
            tgQ = [[dram.tile([NCORES * QR, 128], bf16, name=f"tgQ{k}_{q}",
                              tag=f"tgQ{k}_{q}") for q in range(4)]
                   for k in range(DEPTH)]
            # superwindow after which quarter q's table rows are finalized
            AGSW = {((q + 1) * (QR // 128) - 1) // GW: q for q in range(3)}

            def ag_quarter(e, q):
                nc.gpsimd.collective_compute(
                    "AllGather", AO.bypass,
                    replica_groups=[list(range(NCORES))],
                    ins=[tlgs[e][q * QR:(q + 1) * QR, :].opt()],
                    outs=[tgQ[e][q].opt()])

            def build_S(s):
                """One-hot scatter matrices for superwindow s (bulk build)."""
                cap = plan.sw_cap[s]
                nblk = cap // 128
                o0 = int(plan.sw_off[s])
                maxnblk = max(plan.sw_cap) // 128
                sohc = pool.tile([128, maxnblk], f32, tag="sohc")
                nc.sync.dma_start(sohc[:, :nblk],
                                  sohb[:, o0 // 128:o0 // 128 + nblk])
                S = pool.tile([128, nblk, 128], bf16, tag="S")
                for b in range(nblk):
                    nc.vector.tensor_scalar(S[:, b, :], iota_b[:],
                                            sohc[:, b:b + 1], None,
                                            op0=AO.is_equal)
                return S

            def window_chunks(s, w):
                """(block) list of window w within superwindow s."""
                o = int(plan.sw_off[s])
                b0 = (int(plan.slotQ[w, 0]) - o) // 128
                nb = int(plan.capQ[w].sum()) // 128
                return [b0 + c for c in range(nb)]

            def scatter_sw(s, S, src_tile):
                """One-hot scatter of src_tile rows into per-window psum."""
                pw = psum.tile([128, 512], f32, tag="pw")
                for wl, w in enumerate(plan.sw_windows[s]):
                    chunks = window_chunks(s, w)
                    for ci, b in enumerate(chunks):
                        nc.tensor.matmul(
                            pw[:, wl * 128:wl * 128 + OUT], S[:, b, :],
                            src_tile[:, b, :OUT], start=(ci == 0),
                            stop=(ci == len(chunks) - 1))
                return pw

            def finalize_sw(it, s, pw):
                """pw[:, wl*128:...] holds the scatter sums per window."""
                ws = plan.sw_windows[s]
                nw = len(ws)
                w0 = ws[0]
                hC = pool.tile([128, GW, 128], f32, tag="hC", name="hC")
                ttabs = (pool.tile([128, GW, 256], bf16, tag="ttabs",
                                   name="ttabs")
                         if it < DEPTH else None)
                for wl, w in enumerate(ws):
                    pwv = pw[:, wl * 128:wl * 128 + OUT]
                    nc.vector.tensor_scalar(hC[:, wl, :OUT], pwv,
                                            invdeg_t[:, w:w + 1], None,
                                            op0=AO.mult)
                    if it < DEPTH:
                        pf = psum.tile([128, 512], f32, tag="pfin")
                        nc.tensor.transpose(pf[:OUT, :128], hC[:, wl, :OUT],
                                            ident[:])
                        hT = pool.tile([OUT + 1, 128], bf16, tag="hT")
                        nc.vector.memset(hT[:], 1.0)
                        nc.vector.tensor_copy(hT[:OUT, :], pf[:OUT, :128])
                        nc.tensor.matmul(pf[:, 128:384], hT[:], whsd_t[:],
                                         start=True, stop=True)
                        nc.scalar.copy(ttabs[:, wl, :], pf[:, 128:384])
                rows = slice(w0 * 128, (w0 + nw) * 128)
                if it == 0:
                    nc.sync.dma_start(
                        out_fn[rows, :].rearrange("(b p) f -> p b f", p=128),
                        hC[:, :nw, :OUT])
                if it == DEPTH:
                    nc.sync.dma_start(
                        out_h[rows, :].rearrange("(b p) f -> p b f", p=128),
                        hC[:, :nw, :OUT])
                    return
                nc.sync.dma_start(
                    tlgs[it][rows, :].rearrange("(b p) f -> p b f", p=128),
                    ttabs[:, :nw, 0:128])
                nc.sync.dma_start(
                    tlgd[it][rows, :].rearrange("(b p) f -> p b f", p=128),
                    ttabs[:, :nw, 128:256])

            # ---- pre-pass + iter 0 ----
            for s in range(NSW):
                cap = plan.sw_cap[s]
                nblk = cap // 128
                o0 = int(plan.sw_off[s])
                efm_t = pool.tile([34, cap], bf16, tag="big0")
                nc.sync.dma_start(efm_t[:], efm[:, o0:o0 + cap])
                S = build_S(s)
                feT = pool.tile([OUT, cap], bf16, tag="big1")

                for gi, g0 in enumerate(range(0, cap, 512)):
                    g1 = min(g0 + 512, cap)
                    p1 = psum.tile([128, 512], f32, tag="pA")
                    nc.tensor.matmul(p1[:OUT, :g1 - g0], wiaug[:, :OUT],
                                     efm_t[0:33, g0:g1], start=True, stop=True)
                    if gi % 2 == 0:
                        nc.scalar.activation(feT[:, g0:g1], p1[:OUT, :g1 - g0],
                                             AF.Relu)
                    else:
                        nc.vector.tensor_scalar(feT[:, g0:g1],
                                                p1[:OUT, :g1 - g0], 0.0, None,
                                                op0=AO.max)

                # per-window chunk bookkeeping for interleaved scatter
                block2w = {}
                wlen, wdone = {}, {}
                for wl, w in enumerate(plan.sw_windows[s]):
                    chunks = window_chunks(s, w)
                    wlen[w] = len(chunks)
                    wdone[w] = 0
                    for b in chunks:
                        block2w[b] = (wl, w)
                pw = psum.tile([128, 512], f32, tag="pw")

                for gi, g0 in enumerate(range(0, cap, 512)):
                    g1 = min(g0 + 512, cap)
                    nb = (g1 - g0) // 128
                    pb = psum.tile([128, 512], f32, tag="pbase")
                    pf = psum.tile([128, 512], f32, tag="pA")
                    for ci in range(nb):
                        sl = slice(g0 + ci * 128, g0 + (ci + 1) * 128)
                        cl = slice(ci * 128, (ci + 1) * 128)
                        nc.tensor.matmul(pb[:, cl], feT[:, sl], whmid_t[:],
                                         start=True, stop=False)
                        nc.tensor.matmul(pb[:, cl], efm_t[32:34, sl],
                                         wp2_t[32:34, :], start=False,
                                         stop=True)
                        nc.tensor.matmul(pf[:, cl], efm_t[0:33, sl], wiaug[:],
                                         start=True, stop=True)
                    basec = pool.tile([128, 512], bf16, tag="basec", bufs=3)
                    fee = pool.tile([128, 4, 128], bf16, tag="fee", bufs=3)
                    fv = fee[:].rearrange("p b f -> p (b f)")
                    if gi % 2 == 0:
                        nc.vector.tensor_copy(basec[:, :g1 - g0],
                                              pb[:, :g1 - g0])
                        nc.scalar.activation(fv[:, :g1 - g0], pf[:, :g1 - g0],
                                             AF.Relu)
                    else:
                        nc.scalar.copy(basec[:, :g1 - g0], pb[:, :g1 - g0])
                        nc.vector.tensor_scalar(fv[:, :g1 - g0],
                                                pf[:, :g1 - g0],
                                                0.0, None, op0=AO.max)
                    nc.sync.dma_start(baseH2[:, o0 + g0:o0 + g1],
                                      basec[:, :g1 - g0])
                    for ci in range(nb):
                        b = g0 // 128 + ci
                        wl, w = block2w[b]
                        k = wdone[w]
                        wdone[w] = k + 1
                        nc.tensor.matmul(
                            pw[:, wl * 128:wl * 128 + OUT], S[:, b, :],
                            fee[:, ci, :OUT], start=(k == 0),
                            stop=(k == wlen[w] - 1))
                finalize_sw(0, s, pw)
                if s in AGSW:
                    ag_quarter(0, AGSW[s])
            ag_quarter(0, 3)

            # ---- iterations 1..DEPTH ----
            for it in range(1, DEPTH + 1):
                for s in range(NSW):
                    cap = plan.sw_cap[s]
                    nblk = cap // 128
                    o0 = int(plan.sw_off[s])
                    GS = pool.tile([128, nblk, 128], bf16, tag="big0")
                    GD = pool.tile([128, nblk, 128], bf16, tag="big1")
                    BASE = pool.tile([128, nblk, 128], bf16, tag="big2")
                    nc.sync.dma_start(
                        BASE[:].rearrange("p b f -> p (b f)"),
                        baseH2[:, o0:o0 + cap])
                    S = build_S(s)
                    gsix = pool.tile([128, cap // 16], i16, tag="gsix")
                    nc.sync.dma_start(gsix[:],
                                      gs_idx[:, o0 // 16:(o0 + cap) // 16])
                    gdix = pool.tile([128, cap // 16], i16, tag="gdix")
                    nc.sync.dma_start(gdix[:],
                                      gd_idx[:, o0 // 16:(o0 + cap) // 16])

                    GPC = 1024   # gather piece size (fits the SWDGE ring)
                    for q0 in range(0, cap, GPC):
                        m = min(GPC, cap - q0)
                        nc.gpsimd.dma_gather(
                            GD[:, q0 // 128:(q0 + m) // 128, :],
                            tlgd[it - 1][:], gdix[:, q0 // 16:(q0 + m) // 16],
                            m, m, 128, elem_step=128)
                    for w in plan.sw_windows[s]:
                        regs = [(int(plan.slotQ[w, q]) - o0,
                                 int(plan.capQ[w, q]), tgQ[it - 1][q][:])
                                for q in range(4)]
                        for r0, ln, srcv in regs:
                            for q0 in range(r0, r0 + ln, GPC):
                                m = min(GPC, r0 + ln - q0)
                                nc.gpsimd.dma_gather(
                                    GS[:, q0 // 128:(q0 + m) // 128, :], srcv,
                                    gsix[:, q0 // 16:(q0 + m) // 16],
                                    m, m, 128, elem_step=128)

                    qs = max(1, nblk // 4)
                    bnds = list(range(0, nblk, qs)) + [nblk]
                    for b0, b1 in zip(bnds[:-1], bnds[1:]):
                        gsv = GS[:, b0:b1, :]
                        nc.vector.tensor_tensor(gsv, gsv,
                                                BASE[:, b0:b1, :], AO.add)
                        nc.vector.tensor_tensor(gsv, gsv,
                                                GD[:, b0:b1, :], AO.add)
                        nc.scalar.activation(gsv, gsv, AF.Relu)

                    pw = scatter_sw(s, S, GS)
                    finalize_sw(it, s, pw)
                    if it < DEPTH and s in AGSW:
                        ag_quarter(it, AGSW[s])
                if it < DEPTH:
                    ag_quarter(it, 3)

    nc.compile()
    return nc


_CACHE = {}


def kernel(e, p, gamma, beta, W_i, b_i, W_h, b_h, src, dst, num_nodes):
    e = np.asarray(e, np.float32)
    p = np.asarray(p, np.float32)
    src = np.asarray(src, np.int64)
    dst = np.asarray(dst, np.int64)
    N = int(num_nodes)
    OUT = int(np.asarray(W_i).shape[1])

    plan = Plan(src, dst, N)
    sig = plan.signature()
    if sig not in _CACHE:
        _CACHE[sig] = _build(plan, OUT)
    nc = _CACHE[sig]

    per_core = _host_inputs(plan, e, p, src, dst)
    wts = _weight_inputs(plan, np.asarray(gamma), np.asarray(beta),
                         np.asarray(W_i), np.asarray(b_i),
                         np.asarray(W_h), np.asarray(b_h))
    in_maps = [dict(m, **wts) for m in per_core]

    res = run_bass_kernel_spmd(nc, in_maps, core_ids=list(range(NCORES)))
    fn = np.concatenate([np.asarray(res.results[r]["out_fn"],
                                    np.float32)[:plan.NLOC]
                         for r in range(NCORES)], 0)[:N]
    h = np.concatenate([np.asarray(res.results[r]["out_h"],
                                   np.float32)[:plan.NLOC]
                        for r in range(NCORES)], 0)[:N]
    return np.concatenate([fn, h], axis=1)
